# revision 32
# baseline (speedup 1.0000x reference)
"""Trainium2 Bass kernel for nn_L1RegressionActionHead.

Data-parallel over batch: 16 batch items -> 8 cores x 2 items.
All activations are dim-major on chip: (dims on partitions, tokens on the
free axis), so every matmul streams with the contraction dim on partitions.

RoPE: q/k projection weights are column-permuted on the host so each head's
128 dims are de-interleaved (even dims on partitions 0:64, odd on 64:128).
rotate_half is then a swap of the two 64-partition halves (2 SBUF->SBUF DMAs
issued from the idle gpsimd queue) and cos/sin become plain elementwise
multiplies.  1/sqrt(HD) is folded into the q tables, sigmoid(g) into the
k_task tables, the rotate sign into sin.  Ropes run on PAIRS of n-tiles
(one [128, 2, 1024] op via broadcast tables) to halve DVE op overhead, and
some cos-multiplies go to the otherwise-idle Pool (gpsimd) engine.

The q/o projections run as fp8(e4m3) DoubleRow matmuls (2 k-tiles per pass):
weights and x are quantized host-side (x*32, w*2048), the scale is folded
into the psum-consuming activation, and the o_proj output is carried as
64*y end-to-end (layernorm is scale-invariant; eps is scaled to match).

Softmax: |scores| < ~4 so exp needs no max subtraction.  The denominator is
summed+broadcast with a ones-matrix matmul and the normalization is a single
DVE tensor_tensor divide (psum / psum -> fp8) - no reciprocal roundtrip.

Schedule: the PE is kept saturated front-to-back (TRN2 drops the PE clock
to 1.2GHz after any idle gap and takes ~3us to re-ramp, so every bubble
costs double).  Attention for batch 0 is spread across the v/q_a1 GEMM
phases head-by-head; attention b1 and the LN stats ride under o_proj.
"""

import math
import sys

import numpy as np

sys.path.insert(0, "/opt/trn_rl_repo")

import ml_dtypes  # noqa: E402

import concourse.bass as bass  # noqa: E402
import concourse.tile as tile  # noqa: E402
from concourse import bacc, mybir  # noqa: E402
from concourse.bass_utils import run_bass_kernel_spmd  # noqa: E402

BF16 = ml_dtypes.bfloat16
FP8 = ml_dtypes.float8_e4m3fn  # matches TRN float8e4 bit layout for |v|<=240
F32 = mybir.dt.float32
BF = mybir.dt.bfloat16
AF = mybir.ActivationFunctionType
F8D = mybir.dt.float8e4
OP = mybir.AluOpType

DIM = 1024
HEADS = 8
HD = 128
B = 16
T = 1024
KT = 64
KA = 2
KV = KT + KA  # 66
LN_EPS = 1e-5
NCORES = 8
BPC = B // NCORES  # 2 batch items per core
P = 128
TK = DIM // P  # 8 k/d tiles
NCH = T // 512  # 2 free-dim chunks of 512 tokens
XS8 = 32.0     # fp8 scale for x
WS8 = 2048.0   # fp8 scale for q weights
QSCALE = 1.0 / (XS8 * WS8)  # folded into the q identity activation
OSC = 64.0     # attention-output fp8 scale; y is carried as 64*y (LN-invariant)

# de-interleave: even dims on partitions 0:64, odd dims on 64:128, so
# rotate_half is a swap of the two 64-partition halves (2 SBUF->SBUF DMAs)
# with the sign folded into the sin table.
_PERM_HEAD = np.concatenate([np.arange(0, HD, 2), np.arange(1, HD, 2)])
_SIGN_HEAD = np.concatenate([-np.ones(64, np.float32), np.ones(64, np.float32)])
_PERM_FULL = np.concatenate([h * HD + _PERM_HEAD for h in range(HEADS)])

# weight order inside the "wcat" input tensor
_WIDX = {"w_qa": 0, "w_qt": 1, "w_ka": 2, "w_kt": 3, "w_va": 4, "w_vt": 5,
         "w_o": 6, "w_ffn": 7}
# bias slots inside "bias_cat": per-partition [128, slot, ko]
_BIDX = {"b_qa": 0, "b_qt": 1, "b_ka": 2, "b_kt": 3, "b_o": 4, "b_ffn": 5}

USE_DIVIDE = False    # DVE tensor_tensor divide for softmax/LN normalize
USE_POOL_COS = True  # route some rope cos-muls to the Pool engine
USE_BCAST = True     # stride-0 broadcast rope tables (pair ropes)

_CACHED = None  # compiled Bass program, built once per process
LAST_RESULTS = None  # BassKernelResults of the most recent run


def _build_program():
    nc = bacc.Bacc("TRN2", target_bir_lowering=False, debug=False,
                   enable_asserts=False)

    xt_d = nc.dram_tensor("xt", (P, BPC, TK, T), BF, kind="ExternalInput").ap()
    xt8_d = nc.dram_tensor("xt8", (P, BPC, TK, T), F8D, kind="ExternalInput").ap()
    wq8_d = nc.dram_tensor("wq8", (3, P, TK, DIM), F8D, kind="ExternalInput").ap()
    hcat8_d = nc.dram_tensor("hcat8", (P, TK, 512), F8D, kind="ExternalInput").ap()
    wk8_d = nc.dram_tensor("wk8", (4, P, TK, DIM), F8D, kind="ExternalInput").ap()
    wcat_d = nc.dram_tensor("wcat", (8, P, TK, DIM), BF, kind="ExternalInput").ap()
    bias_d = nc.dram_tensor("bias_cat", (P, 6, TK), F32, kind="ExternalInput").ap()
    bv_d = nc.dram_tensor("bv_comb", (P, DIM), BF, kind="ExternalInput").ap()
    vsel_d = nc.dram_tensor("vsel", (P, P), BF, kind="ExternalInput").ap()
    cosq_d = nc.dram_tensor("cosq", (P, T), BF, kind="ExternalInput").ap()
    sinq_d = nc.dram_tensor("sinq", (P, T), BF, kind="ExternalInput").ap()
    cosk_d = nc.dram_tensor("cosk", (P, 2 * KV), BF, kind="ExternalInput").ap()
    sink_d = nc.dram_tensor("sink", (P, 2 * KV), BF, kind="ExternalInput").ap()
    out_d = nc.dram_tensor("outt", (P, BPC, TK, T), BF, kind="ExternalOutput").ap()

    with tile.TileContext(nc) as tc:
        _trace(nc, tc, xt_d, xt8_d, wq8_d, hcat8_d, wk8_d, wcat_d, bias_d,
               bv_d, vsel_d, cosq_d, sinq_d, cosk_d, sink_d, out_d)
    nc.compile()
    return nc


def _trace(nc, tc, xt_d, xt8_d, wq8_d, hcat8_d, wk8_d, wcat_d, bias_d,
           bv_d, vsel_d, cosq_d, sinq_d, cosk_d, sink_d, out_d):
    import contextlib
    ctx = contextlib.ExitStack()
    with ctx:
        consts = ctx.enter_context(tc.tile_pool(name="consts", bufs=1))
        acts = ctx.enter_context(tc.tile_pool(name="acts", bufs=1))
        qpool = ctx.enter_context(tc.tile_pool(name="qpool", bufs=4))
        wpool = ctx.enter_context(tc.tile_pool(name="wpool", bufs=1))
        wk8p = ctx.enter_context(tc.tile_pool(name="wk8p", bufs=2))
        swp = ctx.enter_context(tc.tile_pool(name="swp", bufs=3))
        q8p = ctx.enter_context(tc.tile_pool(name="q8p", bufs=2))
        sb512 = ctx.enter_context(tc.tile_pool(name="sb512", bufs=2))
        rcp_p = ctx.enter_context(tc.tile_pool(name="rcpp", bufs=1))
        psum = ctx.enter_context(tc.tile_pool(name="psum", bufs=4, space="PSUM"))
        pacc = ctx.enter_context(tc.tile_pool(name="pacc", bufs=2, space="PSUM"))

        def load_w(wname):
            wt = wpool.tile([P, TK, DIM], BF, tag="w", name=wname)
            nc.sync.dma_start(wt[:, :, :], wcat_d[_WIDX[wname], :, :, :])
            return wt

        # ---- DMAs in need order: the q_adapter b0 fp8 GEMM goes first so
        #      the PE starts within ~4us; everything else lands under it.
        # ---- early loads: stripe chunks round-robin over the three
        #      DMA-capable queues (sync/scalar/gpsimd, ~60 GB/s each) in
        #      global need order, so qa0's 2MB is resident by ~19us and
        #      the k projections' weights right behind it.
        _rr = [nc.sync, nc.scalar, nc.gpsimd]
        _rri = [0]

        def rr_dma(dst, srcap):
            _rr[_rri[0] % 3].dma_start(dst, srcap)
            _rri[0] += 1

        bias_sb = consts.tile([P, 6, TK], F32, tag="bias")
        nc.scalar.dma_start(bias_sb[:], bias_d[:])
        wq8a = q8p.tile([P, TK, DIM], F8D, tag="q8", name="wq8a")
        xt8_sb = acts.tile([P, BPC, TK, T], F8D, tag="xt8")
        for k in range(0, TK, 2):
            rr_dma(wq8a[:, k:k + 2, :], wq8_d[0, :, k:k + 2, :])
            rr_dma(xt8_sb[:, 0, k:k + 2, :], xt8_d[:, 0, k:k + 2, :])
        cosq_sb = consts.tile([P, T], BF, tag="cosq")
        rr_dma(cosq_sb[:], cosq_d[:])
        sinq_sb = consts.tile([P, T], BF, tag="sinq")
        rr_dma(sinq_sb[:], sinq_d[:])
        wkt8 = wk8p.tile([P, TK, DIM], F8D, tag="wk8", name="wkt8")
        hcat8 = consts.tile([P, TK, 512], F8D, tag="hcat8")
        rr_dma(hcat8[:, 0:4, :], hcat8_d[:, 0:4, :])
        rr_dma(hcat8[:, 4:8, :], hcat8_d[:, 4:8, :])
        for k in range(0, TK, 2):
            rr_dma(wkt8[:, k:k + 2, :], wk8_d[0, :, k:k + 2, :])
        wka8 = wk8p.tile([P, TK, DIM], F8D, tag="wk8", name="wka8")
        for k in range(0, TK, 2):
            rr_dma(wka8[:, k:k + 2, :], wk8_d[1, :, k:k + 2, :])
        wq8t = q8p.tile([P, TK, DIM], F8D, tag="q8", name="wq8t")
        for k in range(0, TK, 2):
            rr_dma(wq8t[:, k:k + 2, :], wq8_d[1, :, k:k + 2, :])
        cosk_sb = consts.tile([P, 2 * KV], BF, tag="cosk")
        rr_dma(cosk_sb[:], cosk_d[:])
        sink_sb = consts.tile([P, 2 * KV], BF, tag="sink")
        rr_dma(sink_sb[:], sink_d[:])
        bv_sb = consts.tile([P, DIM], BF, tag="bv")
        rr_dma(bv_sb[:], bv_d[:])
        vsel = consts.tile([P, P], BF, tag="vsel")
        rr_dma(vsel[:], vsel_d[:])
        nc.gpsimd.dma_start(xt8_sb[:, 1], xt8_d[:, 1])
        ones_mat = consts.tile([P, P], BF, tag="onesm")
        nc.vector.memset(ones_mat[:], 1.0)
        eps_sb = consts.tile([P, 1], F32, tag="eps")
        nc.vector.memset(eps_sb[:], LN_EPS * OSC * OSC)

        def bias_ap(bname, n):
            return bias_sb[:, _BIDX[bname], n:n + 1]

        DR = mybir.MatmulPerfMode.DoubleRow

        def rope_q(dst, n, pool_cos=False):
            # dst: (128, TK, T) bf16, ropes tile n in place.
            # rotate_half: swap the two 64-partition blocks via 2 DMAs
            # issued from two idle queues; cos-mul optionally on Pool.
            sw = swp.tile([P, T], BF, tag="sw", name=f"sw{n}")
            sl = dst[:, n, :]
            nc.gpsimd.dma_start(sw[0:64, :], dst[64:128, n, :])
            nc.sync.dma_start(sw[64:128, :], dst[0:64, n, :])
            nc.vector.tensor_mul(sw[:], sw[:], sinq_sb[:])
            eng = nc.gpsimd if (pool_cos and USE_POOL_COS) else nc.vector
            eng.tensor_mul(sl, sl, cosq_sb[:])
            nc.vector.tensor_add(sl, sl, sw[:])

        def q_mm(qt_t, w8, bname, b, n, pool_cos=False):
            # fp8 DoubleRow: contract 2 k-tiles per pass (K=256 virtual)
            for c in range(NCH):
                cs = slice(c * 512, (c + 1) * 512)
                ps = psum.tile([P, 512], F32, tag="ps")
                for kp in range(TK // 2):
                    nc.tensor.matmul(
                        ps[:], w8[:, 2 * kp:2 * kp + 2, n * P:(n + 1) * P],
                        xt8_sb[:, b, 2 * kp:2 * kp + 2, cs],
                        start=(kp == 0), stop=(kp == TK // 2 - 1),
                        perf_mode=DR)
                nc.scalar.activation(
                    qt_t[:, n, cs], ps[:], AF.Identity,
                    bias=bias_ap(bname, n), scale=QSCALE)
            rope_q(qt_t, n, pool_cos=pool_cos)

        # ================= q_adapter b0 ================================
        q_rot = {}  # (qi, b) -> (128, TK, T) bf16, qi: 0=adapter 1=task
        qa0 = qpool.tile([P, TK, T], BF, tag="qbuf", name="qa0")
        q_rot[(0, 0)] = qa0
        for n in range(TK):
            q_mm(qa0, wq8a, "b_qa", 0, n)

        # ================= k projections ===============================
        # krot columns: [0:64]=task b0, [64:128]=task b1, [128:130]=ad b0,
        # [130:132]=ad b1
        krot = acts.tile([P, TK, 2 * KV], BF, tag="krot")
        for n in range(TK):
            ps = psum.tile([P, 512], F32, tag="ps")
            for kp in range(TK // 2):
                nc.tensor.matmul(ps[:, 0:128],
                                 wkt8[:, 2 * kp:2 * kp + 2, n * P:(n + 1) * P],
                                 hcat8[:, 2 * kp:2 * kp + 2, 0:128],
                                 start=(kp == 0), stop=(kp == TK // 2 - 1),
                                 perf_mode=DR)
            nc.scalar.activation(krot[:, n, 0:128], ps[:, 0:128],
                                 AF.Identity, bias=bias_ap("b_kt", n),
                                 scale=QSCALE)
        for n in range(TK):
            ps = psum.tile([P, 512], F32, tag="ps")
            for kp in range(TK // 2):
                nc.tensor.matmul(ps[:, 128:132],
                                 wka8[:, 2 * kp:2 * kp + 2, n * P:(n + 1) * P],
                                 hcat8[:, 2 * kp:2 * kp + 2, 192:196],
                                 start=(kp == 0), stop=(kp == TK // 2 - 1),
                                 perf_mode=DR)
            nc.scalar.activation(krot[:, n, 128:132], ps[:, 128:132],
                                 AF.Identity, bias=bias_ap("b_ka", n),
                                 scale=QSCALE)

        # ---- k rope (early: every attention score matmul waits on it) --
        cosk_b = cosk_sb[:].unsqueeze(1).broadcast_to([P, 2, 2 * KV])
        sink_b = sink_sb[:].unsqueeze(1).broadcast_to([P, 2, 2 * KV])
        for n in range(TK):
            sw = rcp_p.tile([P, 2 * KV], BF, tag="ksw", name=f"ksw{n}")
            sl = krot[:, n, :]
            nc.gpsimd.dma_start(sw[0:64, :], krot[64:128, n, :])
            nc.sync.dma_start(sw[64:128, :], krot[0:64, n, :])
            nc.vector.tensor_mul(sw[:], sw[:], sink_sb[:])
            nc.vector.tensor_mul(sl, sl, cosk_sb[:])
            nc.vector.tensor_add(sl, sl, sw[:])

        # v weights into the slots freed by wkt8/wka8 (their k-GEMM reads
        # are already complete here, so the issue does not block the queue);
        # the big bf16 residual tensors follow on the same Activation queue
        # (not needed until o_proj at ~100us)
        wvt8 = wk8p.tile([P, TK, DIM], F8D, tag="wk8", name="wvt8")
        nc.scalar.dma_start(wvt8[:], wk8_d[2])
        wva8 = wk8p.tile([P, TK, DIM], F8D, tag="wk8", name="wva8")
        nc.scalar.dma_start(wva8[:], wk8_d[3])
        xt_sb = acts.tile([P, BPC, TK, T], BF, tag="xt")
        nc.scalar.dma_start(xt_sb[:, 0], xt_d[:, 0])
        nc.scalar.dma_start(xt_sb[:, 1], xt_d[:, 1])

        # ================= q_task b0 ===================================
        qt0 = qpool.tile([P, TK, T], BF, tag="qbuf", name="qt0")
        q_rot[(1, 0)] = qt0
        for n in range(TK):
            q_mm(qt0, wq8t, "b_qt", 0, n)

        # ---- attention helpers ----------------------------------------
        attn = {}
        exs = {}

        def attn_scores(b, h):
            out = []
            for c in range(NCH):
                cs = slice(c * 512, (c + 1) * 512)
                scps = psum.tile([P, 512], F32, tag="ps")
                nc.tensor.matmul(scps[0:64, :], krot[:, h, b * 64:(b + 1) * 64],
                                 q_rot[(1, b)][:, h, cs], start=True, stop=True)
                nc.tensor.matmul(scps[64:66, :],
                                 krot[:, h, 128 + 2 * b:130 + 2 * b],
                                 q_rot[(0, b)][:, h, cs], start=True, stop=True)
                ex = sb512.tile([P, 512], BF, tag="ex", bufs=4, name="ex")
                nc.scalar.activation(ex[0:KV, :], scps[0:KV, :], AF.Exp)
                out.append(ex)
            exs[(b, h)] = out

        def attn_finish(b, h):
            at = attn[b]
            for c in range(NCH):
                cs = slice(c * 512, (c + 1) * 512)
                ex = exs[(b, h)][c]
                dnps = psum.tile([P, 512], F32, tag="ps")
                nc.tensor.matmul(dnps[:], ones_mat[0:KV, :],
                                 ex[0:KV, :], start=True, stop=True)
                ovps = psum.tile([P, 512], F32, tag="ps")
                nc.tensor.matmul(ovps[:], vcomb[0:KV, b, h * P:(h + 1) * P],
                                 ex[0:KV, :], start=True, stop=True)
                if USE_DIVIDE:
                    nc.vector.tensor_tensor(at[:, h, cs], ovps[:], dnps[:],
                                            OP.divide)
                else:
                    rcb = sb512.tile([P, 512], F32, tag="sdf", bufs=1,
                                     name="rcb")
                    nc.vector.reciprocal_approx_fast(rcb[:], dnps[:])
                    nc.vector.tensor_mul(at[:, h, cs], ovps[:], rcb[:])
            del exs[(b, h)]

        # ================= v projections (token-major) =================
        # vcomb rows: [0:64]=task tokens, [64:66]=adapter tokens; the bias
        # lands via a rank-2 matmul (vsel x bv) so the v pipeline only
        # depends on the PE + ScalarE.  Attention b0 scores for the first
        # two heads ride along (their ropes are already done).
        vcomb = acts.tile([P, BPC, DIM], BF, tag="vcomb")
        attn[0] = acts.tile([P, TK, T], F8D, tag="attn0", name="attn0")
        for b in range(BPC):
            for c in range(NCH):
                cs = slice(c * 512, (c + 1) * 512)
                ps = psum.tile([P, 512], F32, tag="ps")
                # bias first: rank-2 matmul starts the psum group over all
                # 128 rows (vsel is pre-scaled by XS8*WS8 on the host so
                # the shared 1/(XS8*WS8) consume scale cancels)
                nc.tensor.matmul(ps[:, :], vsel[0:2, :], bv_sb[0:2, cs],
                                 start=True, stop=False)
                for kp in range(TK // 2):
                    nc.tensor.matmul(ps[0:64, :],
                                     hcat8[:, 2 * kp:2 * kp + 2,
                                           b * 64:(b + 1) * 64],
                                     wvt8[:, 2 * kp:2 * kp + 2, cs],
                                     start=False, stop=False,
                                     perf_mode=DR, skip_group_check=True)
                # adapter last, 128 columns wide at tile_position (0,0)
                # (DR is only valid there): its 2 tokens sit at stationary
                # column base+64 (hcat8 col 192+2b) so they land on psum
                # rows 64:66; all other stationary columns are zero.  The
                # final pass carries the group stop over all 128 rows.
                for kp in range(TK // 2):
                    nc.tensor.matmul(ps[:, :],
                                     hcat8[:, 2 * kp:2 * kp + 2,
                                           128 + 2 * b:256 + 2 * b],
                                     wva8[:, 2 * kp:2 * kp + 2, cs],
                                     start=False, stop=(kp == TK // 2 - 1),
                                     perf_mode=DR,
                                     skip_group_check=(kp != TK // 2 - 1))
                nc.scalar.activation(vcomb[0:KV, b, cs], ps[0:KV, :],
                                     AF.Identity, scale=QSCALE * OSC)
            attn_scores(0, b)  # heads 0 and 1

        # ============ q_adapter b1 (x) attention b0 ====================
        qa1 = qpool.tile([P, TK, T], BF, tag="qbuf", name="qa1")
        q_rot[(0, 1)] = qa1
        for n in range(TK):
            q_mm(qa1, wq8a, "b_qa", 1, n, pool_cos=(n in (3, 7)))
            if 1 <= n <= 6:
                attn_scores(0, n + 1)
            if n >= 1:
                attn_finish(0, n - 1)
        attn_finish(0, TK - 1)

        wo8 = q8p.tile([P, TK, DIM], F8D, tag="q8", name="wo8")
        nc.sync.dma_start(wo8[:], wq8_d[2])  # into wq8a's slot (reads done)

        # ============ q_task b1 ========================================
        qt1 = qpool.tile([P, TK, T], BF, tag="qbuf", name="qt1")
        q_rot[(1, 1)] = qt1
        for n in range(TK):
            q_mm(qt1, wq8t, "b_qt", 1, n, pool_cos=(n in (3, 7)))

        # ============ o_proj b0 (x) attention b1 (x) LN stats b0 =======
        def oproj_tile(b, n, y):
            # fp8 DoubleRow o_proj; psum = OSC*WS8*(attn@wo), so scale by
            # 1/WS8 and add OSC*(x + b_o) (b_o folded into x on the host)
            for c in range(NCH):
                cs = slice(c * 512, (c + 1) * 512)
                ps = psum.tile([P, 512], F32, tag="ps")
                for kp in range(TK // 2):
                    nc.tensor.matmul(
                        ps[:], wo8[:, 2 * kp:2 * kp + 2, n * P:(n + 1) * P],
                        attn[b][:, 2 * kp:2 * kp + 2, cs],
                        start=(kp == 0), stop=(kp == TK // 2 - 1),
                        perf_mode=DR)
                nc.vector.scalar_tensor_tensor(
                    y[:, n, cs], ps[:], 1.0 / WS8,
                    xt_sb[:, b, n, cs], OP.mult, OP.add)

        def ln_stats_tile(sps, qps, y, n):
            for c in range(NCH):
                cs = slice(c * 512, (c + 1) * 512)
                ysq = sb512.tile([P, 512], BF, tag="s", name=f"ysq{n}{c}")
                nc.scalar.activation(ysq[:], y[:, n, cs], AF.Square)
                nc.tensor.matmul(sps[:, cs], ones_mat[:], y[:, n, cs],
                                 start=(n == 0), stop=(n == TK - 1),
                                 skip_group_check=True)
                nc.tensor.matmul(qps[:, cs], ones_mat[:], ysq[:],
                                 start=(n == 0), stop=(n == TK - 1),
                                 skip_group_check=True)

        attn[1] = acts.tile([P, TK, T], F8D, tag="attn1", name="attn1")
        y0 = qpool.tile([P, TK, T], BF, tag="qbuf", name="y0")
        sps0 = pacc.tile([P, T], F32, tag="acc", name="sps0")
        qps0 = pacc.tile([P, T], F32, tag="acc", name="qps0")
        for n in range(TK):
            oproj_tile(0, n, y0)
            if n <= 6:
                attn_scores(1, n)
            if n >= 1:
                attn_finish(1, n - 1)
            if n >= 1:
                ln_stats_tile(sps0, qps0, y0, n - 1)
        attn_scores(1, TK - 1)
        attn_finish(1, TK - 1)
        ln_stats_tile(sps0, qps0, y0, TK - 1)

        wffn = load_w("w_ffn")  # into slot freed by wva

        # ---- layernorm (folded: yn = (y - mu) / sdev) -----------------
        # split: ln_consume reads the psum stat accumulators (freeing them
        # for the next batch); ln_yn_tile normalizes one n-tile.
        ln_bc = {}

        def ln_consume(b, sps, qps):
            # mu_bc/sd_bc: (128, T) bf16, broadcast over partitions
            mu_bc = swp.tile([P, T], BF, tag="sw", name=f"mu{b}")
            sd_bc = swp.tile([P, T], BF, tag="sw", name=f"sd{b}")
            for c in range(NCH):
                cs = slice(c * 512, (c + 1) * 512)
                nc.vector.tensor_scalar_mul(mu_bc[:, cs], sps[:, cs], 1.0 / DIM)
                m2 = sb512.tile([P, 512], BF, tag="s", name=f"m2{b}{c}")
                nc.vector.tensor_mul(m2[:], mu_bc[:, cs], mu_bc[:, cs])
                vq = sb512.tile([P, 512], BF, tag="s", name=f"vq{b}{c}")
                nc.vector.scalar_tensor_tensor(
                    vq[:], qps[:, cs], 1.0 / DIM, m2[:], OP.mult, OP.subtract)
                if USE_DIVIDE:
                    nc.scalar.activation(sd_bc[:, cs], vq[:], AF.Sqrt,
                                         bias=eps_sb[:], scale=1.0)
                else:
                    sdf = sb512.tile([P, 512], F32, tag="sdf", bufs=1,
                                     name=f"sdf{b}{c}")
                    nc.scalar.activation(sdf[:], vq[:], AF.Sqrt,
                                         bias=eps_sb[:], scale=1.0)
                    nc.vector.reciprocal_approx_fast(sdf[:], sdf[:])
                    nc.vector.tensor_scalar_mul(sd_bc[:, cs], sdf[:], 1.0)
            ln_bc[b] = (mu_bc, sd_bc)

        def ln_yn_tile(b, y, yn, n):
            mu_bc, sd_bc = ln_bc[b]
            nc.vector.tensor_tensor(yn[:, n, :], y[:, n, :], mu_bc[:],
                                    OP.subtract)
            if USE_DIVIDE:
                nc.vector.tensor_tensor(yn[:, n, :], yn[:, n, :], sd_bc[:],
                                        OP.divide)
            else:
                nc.vector.tensor_tensor(yn[:, n, :], yn[:, n, :], sd_bc[:],
                                        OP.mult)

        def ffn_tile(b, n, yn):
            for c in range(NCH):
                cs = slice(c * 512, (c + 1) * 512)
                ps = psum.tile([P, 512], F32, tag="ps")
                for k in range(TK):
                    nc.tensor.matmul(ps[:], wffn[:, k, n * P:(n + 1) * P],
                                     yn[:, k, cs],
                                     start=(k == 0), stop=(k == TK - 1))
                ob = sb512.tile([P, 512], BF, tag="s", name=f"ob{b}{n}{c}")
                nc.scalar.activation(ob[:], ps[:], AF.Relu,
                                     bias=bias_ap("b_ffn", n), scale=1.0)
                nc.sync.dma_start(out_d[:, b, n, cs], ob[:])

        # ============ o_proj b1 (x) LN stats b1 (x) ln_apply(0) ========
        # ln_consume(0) first: it is the only reader of sps0/qps0, so the
        # pacc psum banks are free for sps1/qps1 right after.
        ln_consume(0, sps0, qps0)
        y1 = qpool.tile([P, TK, T], BF, tag="qbuf", name="y1")
        yn0 = qpool.tile([P, TK, T], BF, tag="qbuf", name="yn0")
        sps1 = pacc.tile([P, T], F32, tag="acc", name="sps1")
        qps1 = pacc.tile([P, T], F32, tag="acc", name="qps1")
        for n in range(TK):
            oproj_tile(1, n, y1)
            ln_yn_tile(0, y0, yn0, n)
            if n >= 1:
                ln_stats_tile(sps1, qps1, y1, n - 1)
        ln_stats_tile(sps1, qps1, y1, TK - 1)

        # ============ ln_apply(1), ffn b0 ==============================
        ln_consume(1, sps1, qps1)
        yn1 = qpool.tile([P, TK, T], BF, tag="qbuf", name="yn1")
        for n in range(TK):
            ln_yn_tile(1, y1, yn1, n)
            ffn_tile(0, n, yn0)

        # ============ ffn b1 ===========================================
        for n in range(TK):
            ffn_tile(1, n, yn1)


# =====================  host-side preparation  =========================

def _rope_tables(L):
    inv = 1.0 / (10000.0 ** (np.arange(0, HD, 2, dtype=np.float32) / HD))
    freqs = np.arange(L, dtype=np.float32)[:, None] * inv[None, :]
    emb = np.concatenate([freqs, freqs], axis=-1)  # (L, 128)
    return np.cos(emb), np.sin(emb)


def _perm_tables(L, scale):
    cos, sin = _rope_tables(L)  # (L, 128)
    cosP = (cos[:, _PERM_HEAD].T * scale).astype(np.float32)      # (128, L)
    sinN = (sin[:, _PERM_HEAD].T * _SIGN_HEAD[:, None] * scale).astype(np.float32)
    return cosP, sinN


def _w_sb(w, permute):
    # (1024 k, 1024 n) -> (128 p, 8 ko, 1024 n) bf16, optional column perm
    if permute:
        w = w[:, _PERM_FULL]
    return np.ascontiguousarray(
        w.reshape(TK, P, DIM).transpose(1, 0, 2)).astype(BF16)


def _b_slot(bvec, permute):
    if permute:
        bvec = bvec[_PERM_FULL]
    return bvec.reshape(TK, P).T  # (128, 8)


def kernel(**inputs):
    global _CACHED
    if _CACHED is None:
        _CACHED = _build_program()
    nc = _CACHED

    inp = {k: np.asarray(v) for k, v in inputs.items()}
    x = inp["x"].astype(np.float32)
    h_a = inp["h_a"].astype(np.float32)
    h_t = inp["h_t"].astype(np.float32)
    p_in = inp["p"].astype(np.float32)
    ratio = 1.0 / (1.0 + np.exp(-np.float32(inp["g"][0])))  # sigmoid

    # fold layernorm gamma/beta into the ffn weights
    w_ffn = inp["ln_g"].astype(np.float32)[:, None] * inp["w_ffn"].astype(np.float32)
    b_ffn = inp["b_ffn"].astype(np.float32) + (
        inp["ln_b"].astype(np.float32) @ inp["w_ffn"].astype(np.float32))

    # weights (shared across cores)
    wcat = np.stack([
        _w_sb(inp["w_qa"], True), _w_sb(inp["w_qt"], True),
        _w_sb(inp["w_ka"], True), _w_sb(inp["w_kt"], True),
        _w_sb(inp["w_va"], False), _w_sb(inp["w_vt"], False),
        _w_sb(inp["w_o"], False), _w_sb(w_ffn, False)])

    def _w8(w, permute):
        wp = w.astype(np.float32) * WS8
        if permute:
            wp = wp[:, _PERM_FULL]
        wp = np.clip(wp, -240.0, 240.0)
        return np.ascontiguousarray(
            wp.reshape(TK, P, DIM).transpose(1, 0, 2)).astype(FP8)

    wq8 = np.stack([_w8(inp["w_qa"], True), _w8(inp["w_qt"], True),
                    _w8(inp["w_o"], False)])
    wk8 = np.stack([_w8(inp["w_kt"], True), _w8(inp["w_ka"], True),
                    _w8(inp["w_vt"], False), _w8(inp["w_va"], False)])
    bias_cat = np.stack([
        _b_slot(inp["b_qa"], True), _b_slot(inp["b_qt"], True),
        _b_slot(inp["b_ka"], True), _b_slot(inp["b_kt"], True),
        _b_slot(inp["b_o"], False) * 0.0, _b_slot(b_ffn, False)],
        axis=1).astype(np.float32)  # (128, 6slots, 8ko)
    bv_comb = np.zeros((P, DIM), np.float32)
    bv_comb[0, :] = inp["b_vt"]
    bv_comb[1, :] = inp["b_va"]
    bv_comb = bv_comb.astype(BF16)
    # pre-scaled by XS8*WS8: cancels the fp8 consume scale on the v psum
    vsel = np.zeros((P, P), np.float32)
    vsel[0, 0:KT] = XS8 * WS8
    vsel[1, KT:KV] = XS8 * WS8
    vsel = vsel.astype(BF16)

    cosq, sinq = _perm_tables(T, np.float32(1.0 / math.sqrt(HD)))
    coskt, sinkt = _perm_tables(KT, ratio)
    coska, sinka = _perm_tables(KA, np.float32(1.0))
    cosk = np.concatenate([coskt, coskt, coska, coska], axis=1)  # (128, 132)
    sink = np.concatenate([sinkt, sinkt, sinka, sinka], axis=1)

    shared = {
        "wcat": wcat, "wq8": wq8, "wk8": wk8, "bias_cat": bias_cat,
        "bv_comb": bv_comb, "vsel": vsel,
        "cosq": cosq.astype(BF16), "sinq": sinq.astype(BF16),
        "cosk": cosk.astype(BF16), "sink": sink.astype(BF16),
    }

    in_maps = []
    for core in range(NCORES):
        b0 = core * BPC
        xc = x[b0:b0 + BPC]  # (2, 1024, 1024)
        xtf = np.ascontiguousarray(
            xc.reshape(BPC, T, TK, P).transpose(3, 0, 2, 1))  # (128,2,8,1024)
        xtr = xtf + inp["b_o"].astype(np.float32).reshape(TK, P).T[:, None, :, None]
        xt = (xtr * OSC).astype(BF16)  # o_proj residual: OSC*(x + b_o)
        xt8 = np.clip(xtf * XS8, -240.0, 240.0).astype(FP8)
        hcat8 = np.zeros((P, TK, 512), np.float32)
        for b in range(BPC):
            htT = h_t[b0 + b].T.reshape(TK, P, KT).transpose(1, 0, 2)
            hcat8[:, :, b * KT:(b + 1) * KT] = htT
            had = np.stack([h_a[b0 + b, 0], p_in[b0 + b, 0]], axis=1)  # (1024,2)
            hcat8[:, :, 192 + b * KA:192 + (b + 1) * KA] = (
                had.reshape(TK, P, KA).transpose(1, 0, 2))
        hcat8 = np.clip(hcat8 * XS8, -240.0, 240.0)
        in_maps.append({"xt": xt, "xt8": xt8, "hcat8": hcat8.astype(FP8),
                        **shared})

    res = run_bass_kernel_spmd(nc, in_maps, core_ids=list(range(NCORES)))
    global LAST_RESULTS
    LAST_RESULTS = res

    out = np.empty((B, T, DIM), np.float32)
    for core in range(NCORES):
        ot = res.results[core]["outt"]  # (128, 2, 8, 1024) bf16
        out[core * BPC:(core + 1) * BPC] = (
            ot.astype(np.float32).transpose(1, 3, 2, 0).reshape(BPC, T, DIM))
    return out


# revision 33
# speedup vs baseline: 1.0075x; 1.0075x over previous
"""Trainium2 Bass kernel for nn_L1RegressionActionHead.

Data-parallel over batch: 16 batch items -> 8 cores x 2 items.
All activations are dim-major on chip: (dims on partitions, tokens on the
free axis), so every matmul streams with the contraction dim on partitions.

RoPE: q/k projection weights are column-permuted on the host so each head's
128 dims are de-interleaved (even dims on partitions 0:64, odd on 64:128).
rotate_half is then a swap of the two 64-partition halves (2 SBUF->SBUF DMAs
issued from the idle gpsimd queue) and cos/sin become plain elementwise
multiplies.  1/sqrt(HD) is folded into the q tables, sigmoid(g) into the
k_task tables, the rotate sign into sin.  Ropes run on PAIRS of n-tiles
(one [128, 2, 1024] op via broadcast tables) to halve DVE op overhead, and
some cos-multiplies go to the otherwise-idle Pool (gpsimd) engine.

The q/o projections run as fp8(e4m3) DoubleRow matmuls (2 k-tiles per pass):
weights and x are quantized host-side (x*32, w*2048), the scale is folded
into the psum-consuming activation, and the o_proj output is carried as
64*y end-to-end (layernorm is scale-invariant; eps is scaled to match).

Softmax: |scores| < ~4 so exp needs no max subtraction.  The denominator is
summed+broadcast with a ones-matrix matmul and the normalization is a single
DVE tensor_tensor divide (psum / psum -> fp8) - no reciprocal roundtrip.

Schedule: the PE is kept saturated front-to-back (TRN2 drops the PE clock
to 1.2GHz after any idle gap and takes ~3us to re-ramp, so every bubble
costs double).  Attention for batch 0 is spread across the v/q_a1 GEMM
phases head-by-head; attention b1 and the LN stats ride under o_proj.
"""

import math
import sys

import numpy as np

sys.path.insert(0, "/opt/trn_rl_repo")

import ml_dtypes  # noqa: E402

import concourse.bass as bass  # noqa: E402
import concourse.tile as tile  # noqa: E402
from concourse import bacc, mybir  # noqa: E402
from concourse.bass_utils import run_bass_kernel_spmd  # noqa: E402

BF16 = ml_dtypes.bfloat16
FP8 = ml_dtypes.float8_e4m3fn  # matches TRN float8e4 bit layout for |v|<=240
F32 = mybir.dt.float32
BF = mybir.dt.bfloat16
AF = mybir.ActivationFunctionType
F8D = mybir.dt.float8e4
OP = mybir.AluOpType

DIM = 1024
HEADS = 8
HD = 128
B = 16
T = 1024
KT = 64
KA = 2
KV = KT + KA  # 66
LN_EPS = 1e-5
NCORES = 8
BPC = B // NCORES  # 2 batch items per core
P = 128
TK = DIM // P  # 8 k/d tiles
NCH = T // 512  # 2 free-dim chunks of 512 tokens
XS8 = 32.0     # fp8 scale for x
WS8 = 2048.0   # fp8 scale for q weights
QSCALE = 1.0 / (XS8 * WS8)  # folded into the q identity activation
OSC = 64.0     # attention-output fp8 scale; y is carried as 64*y (LN-invariant)

# de-interleave: even dims on partitions 0:64, odd dims on 64:128, so
# rotate_half is a swap of the two 64-partition halves (2 SBUF->SBUF DMAs)
# with the sign folded into the sin table.
_PERM_HEAD = np.concatenate([np.arange(0, HD, 2), np.arange(1, HD, 2)])
_SIGN_HEAD = np.concatenate([-np.ones(64, np.float32), np.ones(64, np.float32)])
_PERM_FULL = np.concatenate([h * HD + _PERM_HEAD for h in range(HEADS)])

# weight order inside the "wcat" input tensor
_WIDX = {"w_qa": 0, "w_qt": 1, "w_ka": 2, "w_kt": 3, "w_va": 4, "w_vt": 5,
         "w_o": 6, "w_ffn": 7}
# bias slots inside "bias_cat": per-partition [128, slot, ko]
_BIDX = {"b_qa": 0, "b_qt": 1, "b_ka": 2, "b_kt": 3, "b_o": 4, "b_ffn": 5}

USE_DIVIDE = False    # DVE tensor_tensor divide for softmax/LN normalize
USE_POOL_COS = True  # route some rope cos-muls to the Pool engine
USE_BCAST = True     # stride-0 broadcast rope tables (pair ropes)

_CACHED = None  # compiled Bass program, built once per process
LAST_RESULTS = None  # BassKernelResults of the most recent run


def _build_program():
    nc = bacc.Bacc("TRN2", target_bir_lowering=False, debug=False,
                   enable_asserts=False)

    xt_d = nc.dram_tensor("xt", (P, BPC, TK, T), BF, kind="ExternalInput").ap()
    xt8_d = nc.dram_tensor("xt8", (P, BPC, TK, T), F8D, kind="ExternalInput").ap()
    wq8_d = nc.dram_tensor("wq8", (3, P, TK, DIM), F8D, kind="ExternalInput").ap()
    hcat8_d = nc.dram_tensor("hcat8", (P, TK, 512), F8D, kind="ExternalInput").ap()
    wk8_d = nc.dram_tensor("wk8", (4, P, TK, DIM), F8D, kind="ExternalInput").ap()
    wcat_d = nc.dram_tensor("wcat", (8, P, TK, DIM), BF, kind="ExternalInput").ap()
    bias_d = nc.dram_tensor("bias_cat", (P, 6, TK), F32, kind="ExternalInput").ap()
    bv_d = nc.dram_tensor("bv_comb", (P, DIM), BF, kind="ExternalInput").ap()
    vsel_d = nc.dram_tensor("vsel", (P, P), BF, kind="ExternalInput").ap()
    cosq_d = nc.dram_tensor("cosq", (P, T), BF, kind="ExternalInput").ap()
    sinq_d = nc.dram_tensor("sinq", (P, T), BF, kind="ExternalInput").ap()
    cosk_d = nc.dram_tensor("cosk", (P, 2 * KV), BF, kind="ExternalInput").ap()
    sink_d = nc.dram_tensor("sink", (P, 2 * KV), BF, kind="ExternalInput").ap()
    out_d = nc.dram_tensor("outt", (P, BPC, TK, T), BF, kind="ExternalOutput").ap()

    with tile.TileContext(nc) as tc:
        _trace(nc, tc, xt_d, xt8_d, wq8_d, hcat8_d, wk8_d, wcat_d, bias_d,
               bv_d, vsel_d, cosq_d, sinq_d, cosk_d, sink_d, out_d)
    nc.compile()
    return nc


def _trace(nc, tc, xt_d, xt8_d, wq8_d, hcat8_d, wk8_d, wcat_d, bias_d,
           bv_d, vsel_d, cosq_d, sinq_d, cosk_d, sink_d, out_d):
    import contextlib
    ctx = contextlib.ExitStack()
    with ctx:
        consts = ctx.enter_context(tc.tile_pool(name="consts", bufs=1))
        acts = ctx.enter_context(tc.tile_pool(name="acts", bufs=1))
        qpool = ctx.enter_context(tc.tile_pool(name="qpool", bufs=4))
        wpool = ctx.enter_context(tc.tile_pool(name="wpool", bufs=1))
        wk8p = ctx.enter_context(tc.tile_pool(name="wk8p", bufs=2))
        swp = ctx.enter_context(tc.tile_pool(name="swp", bufs=3))
        q8p = ctx.enter_context(tc.tile_pool(name="q8p", bufs=2))
        sb512 = ctx.enter_context(tc.tile_pool(name="sb512", bufs=2))
        rcp_p = ctx.enter_context(tc.tile_pool(name="rcpp", bufs=1))
        psum = ctx.enter_context(tc.tile_pool(name="psum", bufs=4, space="PSUM"))
        pacc = ctx.enter_context(tc.tile_pool(name="pacc", bufs=2, space="PSUM"))

        def load_w(wname):
            wt = wpool.tile([P, TK, DIM], BF, tag="w", name=wname)
            nc.sync.dma_start(wt[:, :, :], wcat_d[_WIDX[wname], :, :, :])
            return wt

        # ---- DMAs in need order: the q_adapter b0 fp8 GEMM goes first so
        #      the PE starts within ~4us; everything else lands under it.
        # ---- early loads, one queue per tensor, in need order.
        # sync: wq8a then wka8; scalar: consts+hcat8+wkt8 (then the v
        # weights and xt after the k GEMMs are emitted); gpsimd: xt8+wq8t.
        bias_sb = consts.tile([P, 6, TK], F32, tag="bias")
        nc.scalar.dma_start(bias_sb[:], bias_d[:])
        cosq_sb = consts.tile([P, T], BF, tag="cosq")
        nc.scalar.dma_start(cosq_sb[:], cosq_d[:])
        sinq_sb = consts.tile([P, T], BF, tag="sinq")
        nc.scalar.dma_start(sinq_sb[:], sinq_d[:])
        wq8a = q8p.tile([P, TK, DIM], F8D, tag="q8", name="wq8a")
        xt8_sb = acts.tile([P, BPC, TK, T], F8D, tag="xt8")
        for k in range(0, TK, 2):
            nc.sync.dma_start(wq8a[:, k:k + 2, :], wq8_d[0, :, k:k + 2, :])
            nc.gpsimd.dma_start(xt8_sb[:, 0, k:k + 2, :],
                                xt8_d[:, 0, k:k + 2, :])
        hcat8 = consts.tile([P, TK, 512], F8D, tag="hcat8")
        nc.scalar.dma_start(hcat8[:], hcat8_d[:])
        wkt8 = wk8p.tile([P, TK, DIM], F8D, tag="wk8", name="wkt8")
        for k in range(0, TK, 2):
            nc.scalar.dma_start(wkt8[:, k:k + 2, :], wk8_d[0, :, k:k + 2, :])
        wka8 = wk8p.tile([P, TK, DIM], F8D, tag="wk8", name="wka8")
        nc.sync.dma_start(wka8[:], wk8_d[1])
        wq8t = q8p.tile([P, TK, DIM], F8D, tag="q8", name="wq8t")
        for k in range(0, TK, 2):
            nc.gpsimd.dma_start(wq8t[:, k:k + 2, :], wq8_d[1, :, k:k + 2, :])
        cosk_sb = consts.tile([P, 2 * KV], BF, tag="cosk")
        nc.scalar.dma_start(cosk_sb[:], cosk_d[:])
        sink_sb = consts.tile([P, 2 * KV], BF, tag="sink")
        nc.scalar.dma_start(sink_sb[:], sink_d[:])
        bv_sb = consts.tile([P, DIM], BF, tag="bv")
        nc.scalar.dma_start(bv_sb[:], bv_d[:])
        vsel = consts.tile([P, P], BF, tag="vsel")
        nc.scalar.dma_start(vsel[:], vsel_d[:])
        nc.gpsimd.dma_start(xt8_sb[:, 1], xt8_d[:, 1])
        ones_mat = consts.tile([P, P], BF, tag="onesm")
        nc.vector.memset(ones_mat[:], 1.0)
        eps_sb = consts.tile([P, 1], F32, tag="eps")
        nc.vector.memset(eps_sb[:], LN_EPS * OSC * OSC)

        def bias_ap(bname, n):
            return bias_sb[:, _BIDX[bname], n:n + 1]

        DR = mybir.MatmulPerfMode.DoubleRow

        def rope_q(dst, n, pool_cos=False):
            # dst: (128, TK, T) bf16, ropes tile n in place.
            # rotate_half: swap the two 64-partition blocks via 2 DMAs
            # issued from two idle queues; cos-mul optionally on Pool.
            sw = swp.tile([P, T], BF, tag="sw", name=f"sw{n}")
            sl = dst[:, n, :]
            nc.gpsimd.dma_start(sw[0:64, :], dst[64:128, n, :])
            nc.sync.dma_start(sw[64:128, :], dst[0:64, n, :])
            nc.vector.tensor_mul(sw[:], sw[:], sinq_sb[:])
            eng = nc.gpsimd if (pool_cos and USE_POOL_COS) else nc.vector
            eng.tensor_mul(sl, sl, cosq_sb[:])
            nc.vector.tensor_add(sl, sl, sw[:])

        def q_mm(qt_t, w8, bname, b, n, pool_cos=False):
            # fp8 DoubleRow: contract 2 k-tiles per pass (K=256 virtual)
            for c in range(NCH):
                cs = slice(c * 512, (c + 1) * 512)
                ps = psum.tile([P, 512], F32, tag="ps")
                for kp in range(TK // 2):
                    nc.tensor.matmul(
                        ps[:], w8[:, 2 * kp:2 * kp + 2, n * P:(n + 1) * P],
                        xt8_sb[:, b, 2 * kp:2 * kp + 2, cs],
                        start=(kp == 0), stop=(kp == TK // 2 - 1),
                        perf_mode=DR)
                nc.scalar.activation(
                    qt_t[:, n, cs], ps[:], AF.Identity,
                    bias=bias_ap(bname, n), scale=QSCALE)
            rope_q(qt_t, n, pool_cos=pool_cos)

        # ================= q_adapter b0 ================================
        q_rot = {}  # (qi, b) -> (128, TK, T) bf16, qi: 0=adapter 1=task
        qa0 = qpool.tile([P, TK, T], BF, tag="qbuf", name="qa0")
        q_rot[(0, 0)] = qa0
        for n in range(TK):
            q_mm(qa0, wq8a, "b_qa", 0, n)

        # ================= k projections ===============================
        # krot columns: [0:64]=task b0, [64:128]=task b1, [128:130]=ad b0,
        # [130:132]=ad b1
        krot = acts.tile([P, TK, 2 * KV], BF, tag="krot")
        for n in range(TK):
            ps = psum.tile([P, 512], F32, tag="ps")
            for kp in range(TK // 2):
                nc.tensor.matmul(ps[:, 0:128],
                                 wkt8[:, 2 * kp:2 * kp + 2, n * P:(n + 1) * P],
                                 hcat8[:, 2 * kp:2 * kp + 2, 0:128],
                                 start=(kp == 0), stop=(kp == TK // 2 - 1),
                                 perf_mode=DR)
            nc.scalar.activation(krot[:, n, 0:128], ps[:, 0:128],
                                 AF.Identity, bias=bias_ap("b_kt", n),
                                 scale=QSCALE)
        for n in range(TK):
            ps = psum.tile([P, 512], F32, tag="ps")
            for kp in range(TK // 2):
                nc.tensor.matmul(ps[:, 128:132],
                                 wka8[:, 2 * kp:2 * kp + 2, n * P:(n + 1) * P],
                                 hcat8[:, 2 * kp:2 * kp + 2, 192:196],
                                 start=(kp == 0), stop=(kp == TK // 2 - 1),
                                 perf_mode=DR)
            nc.scalar.activation(krot[:, n, 128:132], ps[:, 128:132],
                                 AF.Identity, bias=bias_ap("b_ka", n),
                                 scale=QSCALE)

        # ---- k rope (early: every attention score matmul waits on it) --
        cosk_b = cosk_sb[:].unsqueeze(1).broadcast_to([P, 2, 2 * KV])
        sink_b = sink_sb[:].unsqueeze(1).broadcast_to([P, 2, 2 * KV])
        for n in range(TK):
            sw = rcp_p.tile([P, 2 * KV], BF, tag="ksw", name=f"ksw{n}")
            sl = krot[:, n, :]
            nc.gpsimd.dma_start(sw[0:64, :], krot[64:128, n, :])
            nc.sync.dma_start(sw[64:128, :], krot[0:64, n, :])
            nc.vector.tensor_mul(sw[:], sw[:], sink_sb[:])
            nc.vector.tensor_mul(sl, sl, cosk_sb[:])
            nc.vector.tensor_add(sl, sl, sw[:])

        # v weights into the slots freed by wkt8/wka8 (their k-GEMM reads
        # are already complete here, so the issue does not block the queue);
        # the big bf16 residual tensors follow on the same Activation queue
        # (not needed until o_proj at ~100us)
        wvt8 = wk8p.tile([P, TK, DIM], F8D, tag="wk8", name="wvt8")
        nc.scalar.dma_start(wvt8[:], wk8_d[2])
        wva8 = wk8p.tile([P, TK, DIM], F8D, tag="wk8", name="wva8")
        nc.scalar.dma_start(wva8[:], wk8_d[3])
        xt_sb = acts.tile([P, BPC, TK, T], BF, tag="xt")
        nc.scalar.dma_start(xt_sb[:, 0], xt_d[:, 0])
        nc.scalar.dma_start(xt_sb[:, 1], xt_d[:, 1])

        # ================= q_task b0 ===================================
        qt0 = qpool.tile([P, TK, T], BF, tag="qbuf", name="qt0")
        q_rot[(1, 0)] = qt0
        for n in range(TK):
            q_mm(qt0, wq8t, "b_qt", 0, n)

        # ---- attention helpers ----------------------------------------
        attn = {}
        exs = {}

        def attn_scores(b, h):
            out = []
            for c in range(NCH):
                cs = slice(c * 512, (c + 1) * 512)
                scps = psum.tile([P, 512], F32, tag="ps")
                nc.tensor.matmul(scps[0:64, :], krot[:, h, b * 64:(b + 1) * 64],
                                 q_rot[(1, b)][:, h, cs], start=True, stop=True)
                nc.tensor.matmul(scps[64:66, :],
                                 krot[:, h, 128 + 2 * b:130 + 2 * b],
                                 q_rot[(0, b)][:, h, cs], start=True, stop=True)
                ex = sb512.tile([P, 512], BF, tag="ex", bufs=4, name="ex")
                nc.scalar.activation(ex[0:KV, :], scps[0:KV, :], AF.Exp)
                out.append(ex)
            exs[(b, h)] = out

        def attn_finish(b, h):
            at = attn[b]
            for c in range(NCH):
                cs = slice(c * 512, (c + 1) * 512)
                ex = exs[(b, h)][c]
                dnps = psum.tile([P, 512], F32, tag="ps")
                nc.tensor.matmul(dnps[:], ones_mat[0:KV, :],
                                 ex[0:KV, :], start=True, stop=True)
                ovps = psum.tile([P, 512], F32, tag="ps")
                nc.tensor.matmul(ovps[:], vcomb[0:KV, b, h * P:(h + 1) * P],
                                 ex[0:KV, :], start=True, stop=True)
                if USE_DIVIDE:
                    nc.vector.tensor_tensor(at[:, h, cs], ovps[:], dnps[:],
                                            OP.divide)
                else:
                    rcb = sb512.tile([P, 512], F32, tag="sdf", bufs=1,
                                     name="rcb")
                    nc.vector.reciprocal_approx_fast(rcb[:], dnps[:])
                    nc.vector.tensor_mul(at[:, h, cs], ovps[:], rcb[:])
            del exs[(b, h)]

        # ================= v projections (token-major) =================
        # vcomb rows: [0:64]=task tokens, [64:66]=adapter tokens; the bias
        # lands via a rank-2 matmul (vsel x bv) so the v pipeline only
        # depends on the PE + ScalarE.  Attention b0 scores for the first
        # two heads ride along (their ropes are already done).
        vcomb = acts.tile([P, BPC, DIM], BF, tag="vcomb")
        attn[0] = acts.tile([P, TK, T], F8D, tag="attn0", name="attn0")
        for b in range(BPC):
            for c in range(NCH):
                cs = slice(c * 512, (c + 1) * 512)
                ps = psum.tile([P, 512], F32, tag="ps")
                # bias first: rank-2 matmul starts the psum group over all
                # 128 rows (vsel is pre-scaled by XS8*WS8 on the host so
                # the shared 1/(XS8*WS8) consume scale cancels)
                nc.tensor.matmul(ps[:, :], vsel[0:2, :], bv_sb[0:2, cs],
                                 start=True, stop=False)
                for kp in range(TK // 2):
                    nc.tensor.matmul(ps[0:64, :],
                                     hcat8[:, 2 * kp:2 * kp + 2,
                                           b * 64:(b + 1) * 64],
                                     wvt8[:, 2 * kp:2 * kp + 2, cs],
                                     start=False, stop=False,
                                     perf_mode=DR, skip_group_check=True)
                # adapter last, 128 columns wide at tile_position (0,0)
                # (DR is only valid there): its 2 tokens sit at stationary
                # column base+64 (hcat8 col 192+2b) so they land on psum
                # rows 64:66; all other stationary columns are zero.  The
                # final pass carries the group stop over all 128 rows.
                for kp in range(TK // 2):
                    nc.tensor.matmul(ps[:, :],
                                     hcat8[:, 2 * kp:2 * kp + 2,
                                           128 + 2 * b:256 + 2 * b],
                                     wva8[:, 2 * kp:2 * kp + 2, cs],
                                     start=False, stop=(kp == TK // 2 - 1),
                                     perf_mode=DR,
                                     skip_group_check=(kp != TK // 2 - 1))
                nc.scalar.activation(vcomb[0:KV, b, cs], ps[0:KV, :],
                                     AF.Identity, scale=QSCALE * OSC)
            attn_scores(0, b)  # heads 0 and 1

        # ============ q_adapter b1 (x) attention b0 ====================
        qa1 = qpool.tile([P, TK, T], BF, tag="qbuf", name="qa1")
        q_rot[(0, 1)] = qa1
        for n in range(TK):
            q_mm(qa1, wq8a, "b_qa", 1, n, pool_cos=(n in (3, 7)))
            if 1 <= n <= 6:
                attn_scores(0, n + 1)
            if n >= 1:
                attn_finish(0, n - 1)
        attn_finish(0, TK - 1)

        wo8 = q8p.tile([P, TK, DIM], F8D, tag="q8", name="wo8")
        nc.sync.dma_start(wo8[:], wq8_d[2])  # into wq8a's slot (reads done)

        # ============ q_task b1 ========================================
        qt1 = qpool.tile([P, TK, T], BF, tag="qbuf", name="qt1")
        q_rot[(1, 1)] = qt1
        for n in range(TK):
            q_mm(qt1, wq8t, "b_qt", 1, n, pool_cos=(n in (3, 7)))

        # ============ o_proj b0 (x) attention b1 (x) LN stats b0 =======
        def oproj_tile(b, n, y):
            # fp8 DoubleRow o_proj; psum = OSC*WS8*(attn@wo), so scale by
            # 1/WS8 and add OSC*(x + b_o) (b_o folded into x on the host)
            for c in range(NCH):
                cs = slice(c * 512, (c + 1) * 512)
                ps = psum.tile([P, 512], F32, tag="ps")
                for kp in range(TK // 2):
                    nc.tensor.matmul(
                        ps[:], wo8[:, 2 * kp:2 * kp + 2, n * P:(n + 1) * P],
                        attn[b][:, 2 * kp:2 * kp + 2, cs],
                        start=(kp == 0), stop=(kp == TK // 2 - 1),
                        perf_mode=DR)
                nc.vector.scalar_tensor_tensor(
                    y[:, n, cs], ps[:], 1.0 / WS8,
                    xt_sb[:, b, n, cs], OP.mult, OP.add)

        def ln_stats_tile(sps, qps, y, n):
            for c in range(NCH):
                cs = slice(c * 512, (c + 1) * 512)
                ysq = sb512.tile([P, 512], BF, tag="s", name=f"ysq{n}{c}")
                nc.scalar.activation(ysq[:], y[:, n, cs], AF.Square)
                nc.tensor.matmul(sps[:, cs], ones_mat[:], y[:, n, cs],
                                 start=(n == 0), stop=(n == TK - 1),
                                 skip_group_check=True)
                nc.tensor.matmul(qps[:, cs], ones_mat[:], ysq[:],
                                 start=(n == 0), stop=(n == TK - 1),
                                 skip_group_check=True)

        attn[1] = acts.tile([P, TK, T], F8D, tag="attn1", name="attn1")
        y0 = qpool.tile([P, TK, T], BF, tag="qbuf", name="y0")
        sps0 = pacc.tile([P, T], F32, tag="acc", name="sps0")
        qps0 = pacc.tile([P, T], F32, tag="acc", name="qps0")
        for n in range(TK):
            oproj_tile(0, n, y0)
            if n <= 6:
                attn_scores(1, n)
            if n >= 1:
                attn_finish(1, n - 1)
            if n >= 1:
                ln_stats_tile(sps0, qps0, y0, n - 1)
        attn_scores(1, TK - 1)
        attn_finish(1, TK - 1)
        ln_stats_tile(sps0, qps0, y0, TK - 1)

        wffn = load_w("w_ffn")  # into slot freed by wva

        # ---- layernorm (folded: yn = (y - mu) / sdev) -----------------
        # split: ln_consume reads the psum stat accumulators (freeing them
        # for the next batch); ln_yn_tile normalizes one n-tile.
        ln_bc = {}

        def ln_consume(b, sps, qps):
            # mu_bc/sd_bc: (128, T) bf16, broadcast over partitions
            mu_bc = swp.tile([P, T], BF, tag="sw", name=f"mu{b}")
            sd_bc = swp.tile([P, T], BF, tag="sw", name=f"sd{b}")
            for c in range(NCH):
                cs = slice(c * 512, (c + 1) * 512)
                nc.vector.tensor_scalar_mul(mu_bc[:, cs], sps[:, cs], 1.0 / DIM)
                m2 = sb512.tile([P, 512], BF, tag="s", name=f"m2{b}{c}")
                nc.vector.tensor_mul(m2[:], mu_bc[:, cs], mu_bc[:, cs])
                vq = sb512.tile([P, 512], BF, tag="s", name=f"vq{b}{c}")
                nc.vector.scalar_tensor_tensor(
                    vq[:], qps[:, cs], 1.0 / DIM, m2[:], OP.mult, OP.subtract)
                if USE_DIVIDE:
                    nc.scalar.activation(sd_bc[:, cs], vq[:], AF.Sqrt,
                                         bias=eps_sb[:], scale=1.0)
                else:
                    sdf = sb512.tile([P, 512], F32, tag="sdf", bufs=1,
                                     name=f"sdf{b}{c}")
                    nc.scalar.activation(sdf[:], vq[:], AF.Sqrt,
                                         bias=eps_sb[:], scale=1.0)
                    nc.vector.reciprocal_approx_fast(sdf[:], sdf[:])
                    nc.vector.tensor_scalar_mul(sd_bc[:, cs], sdf[:], 1.0)
            ln_bc[b] = (mu_bc, sd_bc)

        def ln_yn_tile(b, y, yn, n):
            mu_bc, sd_bc = ln_bc[b]
            nc.vector.tensor_tensor(yn[:, n, :], y[:, n, :], mu_bc[:],
                                    OP.subtract)
            if USE_DIVIDE:
                nc.vector.tensor_tensor(yn[:, n, :], yn[:, n, :], sd_bc[:],
                                        OP.divide)
            else:
                nc.vector.tensor_tensor(yn[:, n, :], yn[:, n, :], sd_bc[:],
                                        OP.mult)

        def ffn_tile(b, n, yn):
            for c in range(NCH):
                cs = slice(c * 512, (c + 1) * 512)
                ps = psum.tile([P, 512], F32, tag="ps")
                for k in range(TK):
                    nc.tensor.matmul(ps[:], wffn[:, k, n * P:(n + 1) * P],
                                     yn[:, k, cs],
                                     start=(k == 0), stop=(k == TK - 1))
                ob = sb512.tile([P, 512], BF, tag="s", name=f"ob{b}{n}{c}")
                nc.scalar.activation(ob[:], ps[:], AF.Relu,
                                     bias=bias_ap("b_ffn", n), scale=1.0)
                nc.sync.dma_start(out_d[:, b, n, cs], ob[:])

        # ============ o_proj b1 (x) LN stats b1 (x) ln_apply(0) ========
        # ln_consume(0) first: it is the only reader of sps0/qps0, so the
        # pacc psum banks are free for sps1/qps1 right after.
        ln_consume(0, sps0, qps0)
        y1 = qpool.tile([P, TK, T], BF, tag="qbuf", name="y1")
        yn0 = qpool.tile([P, TK, T], BF, tag="qbuf", name="yn0")
        sps1 = pacc.tile([P, T], F32, tag="acc", name="sps1")
        qps1 = pacc.tile([P, T], F32, tag="acc", name="qps1")
        for n in range(TK):
            oproj_tile(1, n, y1)
            ln_yn_tile(0, y0, yn0, n)
            if n >= 1:
                ln_stats_tile(sps1, qps1, y1, n - 1)
        ln_stats_tile(sps1, qps1, y1, TK - 1)

        # ============ ln_apply(1), ffn b0 ==============================
        ln_consume(1, sps1, qps1)
        yn1 = qpool.tile([P, TK, T], BF, tag="qbuf", name="yn1")
        for n in range(TK):
            ln_yn_tile(1, y1, yn1, n)
            ffn_tile(0, n, yn0)

        # ============ ffn b1 ===========================================
        for n in range(TK):
            ffn_tile(1, n, yn1)


# =====================  host-side preparation  =========================

def _rope_tables(L):
    inv = 1.0 / (10000.0 ** (np.arange(0, HD, 2, dtype=np.float32) / HD))
    freqs = np.arange(L, dtype=np.float32)[:, None] * inv[None, :]
    emb = np.concatenate([freqs, freqs], axis=-1)  # (L, 128)
    return np.cos(emb), np.sin(emb)


def _perm_tables(L, scale):
    cos, sin = _rope_tables(L)  # (L, 128)
    cosP = (cos[:, _PERM_HEAD].T * scale).astype(np.float32)      # (128, L)
    sinN = (sin[:, _PERM_HEAD].T * _SIGN_HEAD[:, None] * scale).astype(np.float32)
    return cosP, sinN


def _w_sb(w, permute):
    # (1024 k, 1024 n) -> (128 p, 8 ko, 1024 n) bf16, optional column perm
    if permute:
        w = w[:, _PERM_FULL]
    return np.ascontiguousarray(
        w.reshape(TK, P, DIM).transpose(1, 0, 2)).astype(BF16)


def _b_slot(bvec, permute):
    if permute:
        bvec = bvec[_PERM_FULL]
    return bvec.reshape(TK, P).T  # (128, 8)


def kernel(**inputs):
    global _CACHED
    if _CACHED is None:
        _CACHED = _build_program()
    nc = _CACHED

    inp = {k: np.asarray(v) for k, v in inputs.items()}
    x = inp["x"].astype(np.float32)
    h_a = inp["h_a"].astype(np.float32)
    h_t = inp["h_t"].astype(np.float32)
    p_in = inp["p"].astype(np.float32)
    ratio = 1.0 / (1.0 + np.exp(-np.float32(inp["g"][0])))  # sigmoid

    # fold layernorm gamma/beta into the ffn weights
    w_ffn = inp["ln_g"].astype(np.float32)[:, None] * inp["w_ffn"].astype(np.float32)
    b_ffn = inp["b_ffn"].astype(np.float32) + (
        inp["ln_b"].astype(np.float32) @ inp["w_ffn"].astype(np.float32))

    # weights (shared across cores)
    wcat = np.stack([
        _w_sb(inp["w_qa"], True), _w_sb(inp["w_qt"], True),
        _w_sb(inp["w_ka"], True), _w_sb(inp["w_kt"], True),
        _w_sb(inp["w_va"], False), _w_sb(inp["w_vt"], False),
        _w_sb(inp["w_o"], False), _w_sb(w_ffn, False)])

    def _w8(w, permute):
        wp = w.astype(np.float32) * WS8
        if permute:
            wp = wp[:, _PERM_FULL]
        wp = np.clip(wp, -240.0, 240.0)
        return np.ascontiguousarray(
            wp.reshape(TK, P, DIM).transpose(1, 0, 2)).astype(FP8)

    wq8 = np.stack([_w8(inp["w_qa"], True), _w8(inp["w_qt"], True),
                    _w8(inp["w_o"], False)])
    wk8 = np.stack([_w8(inp["w_kt"], True), _w8(inp["w_ka"], True),
                    _w8(inp["w_vt"], False), _w8(inp["w_va"], False)])
    bias_cat = np.stack([
        _b_slot(inp["b_qa"], True), _b_slot(inp["b_qt"], True),
        _b_slot(inp["b_ka"], True), _b_slot(inp["b_kt"], True),
        _b_slot(inp["b_o"], False) * 0.0, _b_slot(b_ffn, False)],
        axis=1).astype(np.float32)  # (128, 6slots, 8ko)
    bv_comb = np.zeros((P, DIM), np.float32)
    bv_comb[0, :] = inp["b_vt"]
    bv_comb[1, :] = inp["b_va"]
    bv_comb = bv_comb.astype(BF16)
    # pre-scaled by XS8*WS8: cancels the fp8 consume scale on the v psum
    vsel = np.zeros((P, P), np.float32)
    vsel[0, 0:KT] = XS8 * WS8
    vsel[1, KT:KV] = XS8 * WS8
    vsel = vsel.astype(BF16)

    cosq, sinq = _perm_tables(T, np.float32(1.0 / math.sqrt(HD)))
    coskt, sinkt = _perm_tables(KT, ratio)
    coska, sinka = _perm_tables(KA, np.float32(1.0))
    cosk = np.concatenate([coskt, coskt, coska, coska], axis=1)  # (128, 132)
    sink = np.concatenate([sinkt, sinkt, sinka, sinka], axis=1)

    shared = {
        "wcat": wcat, "wq8": wq8, "wk8": wk8, "bias_cat": bias_cat,
        "bv_comb": bv_comb, "vsel": vsel,
        "cosq": cosq.astype(BF16), "sinq": sinq.astype(BF16),
        "cosk": cosk.astype(BF16), "sink": sink.astype(BF16),
    }

    in_maps = []
    for core in range(NCORES):
        b0 = core * BPC
        xc = x[b0:b0 + BPC]  # (2, 1024, 1024)
        xtf = np.ascontiguousarray(
            xc.reshape(BPC, T, TK, P).transpose(3, 0, 2, 1))  # (128,2,8,1024)
        xtr = xtf + inp["b_o"].astype(np.float32).reshape(TK, P).T[:, None, :, None]
        xt = (xtr * OSC).astype(BF16)  # o_proj residual: OSC*(x + b_o)
        xt8 = np.clip(xtf * XS8, -240.0, 240.0).astype(FP8)
        hcat8 = np.zeros((P, TK, 512), np.float32)
        for b in range(BPC):
            htT = h_t[b0 + b].T.reshape(TK, P, KT).transpose(1, 0, 2)
            hcat8[:, :, b * KT:(b + 1) * KT] = htT
            had = np.stack([h_a[b0 + b, 0], p_in[b0 + b, 0]], axis=1)  # (1024,2)
            hcat8[:, :, 192 + b * KA:192 + (b + 1) * KA] = (
                had.reshape(TK, P, KA).transpose(1, 0, 2))
        hcat8 = np.clip(hcat8 * XS8, -240.0, 240.0)
        in_maps.append({"xt": xt, "xt8": xt8, "hcat8": hcat8.astype(FP8),
                        **shared})

    res = run_bass_kernel_spmd(nc, in_maps, core_ids=list(range(NCORES)))
    global LAST_RESULTS
    LAST_RESULTS = res

    out = np.empty((B, T, DIM), np.float32)
    for core in range(NCORES):
        ot = res.results[core]["outt"]  # (128, 2, 8, 1024) bf16
        out[core * BPC:(core + 1) * BPC] = (
            ot.astype(np.float32).transpose(1, 3, 2, 0).reshape(BPC, T, DIM))
    return out


# revision 34
# speedup vs baseline: 1.0250x; 1.0174x over previous
"""Trainium2 Bass kernel for nn_L1RegressionActionHead.

Data-parallel over batch: 16 batch items -> 8 cores x 2 items.
All activations are dim-major on chip: (dims on partitions, tokens on the
free axis), so every matmul streams with the contraction dim on partitions.

RoPE: q/k projection weights are column-permuted on the host so each head's
128 dims are de-interleaved (even dims on partitions 0:64, odd on 64:128).
rotate_half is then a swap of the two 64-partition halves (2 SBUF->SBUF DMAs
issued from the idle gpsimd queue) and cos/sin become plain elementwise
multiplies.  1/sqrt(HD) is folded into the q tables, sigmoid(g) into the
k_task tables, the rotate sign into sin.  Ropes run on PAIRS of n-tiles
(one [128, 2, 1024] op via broadcast tables) to halve DVE op overhead, and
some cos-multiplies go to the otherwise-idle Pool (gpsimd) engine.

The q/o projections run as fp8(e4m3) DoubleRow matmuls (2 k-tiles per pass):
weights and x are quantized host-side (x*32, w*2048), the scale is folded
into the psum-consuming activation, and the o_proj output is carried as
64*y end-to-end (layernorm is scale-invariant; eps is scaled to match).

Softmax: |scores| < ~4 so exp needs no max subtraction.  The denominator is
summed+broadcast with a ones-matrix matmul and the normalization is a single
DVE tensor_tensor divide (psum / psum -> fp8) - no reciprocal roundtrip.

Schedule: the PE is kept saturated front-to-back (TRN2 drops the PE clock
to 1.2GHz after any idle gap and takes ~3us to re-ramp, so every bubble
costs double).  Attention for batch 0 is spread across the v/q_a1 GEMM
phases head-by-head; attention b1 and the LN stats ride under o_proj.
"""

import math
import sys

import numpy as np

sys.path.insert(0, "/opt/trn_rl_repo")

import ml_dtypes  # noqa: E402

import concourse.bass as bass  # noqa: E402
import concourse.tile as tile  # noqa: E402
from concourse import bacc, mybir  # noqa: E402
from concourse.bass_utils import run_bass_kernel_spmd  # noqa: E402

BF16 = ml_dtypes.bfloat16
FP8 = ml_dtypes.float8_e4m3fn  # matches TRN float8e4 bit layout for |v|<=240
F32 = mybir.dt.float32
BF = mybir.dt.bfloat16
AF = mybir.ActivationFunctionType
F8D = mybir.dt.float8e4
OP = mybir.AluOpType

DIM = 1024
HEADS = 8
HD = 128
B = 16
T = 1024
KT = 64
KA = 2
KV = KT + KA  # 66
LN_EPS = 1e-5
NCORES = 8
BPC = B // NCORES  # 2 batch items per core
P = 128
TK = DIM // P  # 8 k/d tiles
NCH = T // 512  # 2 free-dim chunks of 512 tokens
XS8 = 32.0     # fp8 scale for x
WS8 = 2048.0   # fp8 scale for q weights
QSCALE = 1.0 / (XS8 * WS8)  # folded into the q identity activation
OSC = 64.0     # attention-output fp8 scale; y is carried as 64*y (LN-invariant)

# de-interleave: even dims on partitions 0:64, odd dims on 64:128, so
# rotate_half is a swap of the two 64-partition halves (2 SBUF->SBUF DMAs)
# with the sign folded into the sin table.
_PERM_HEAD = np.concatenate([np.arange(0, HD, 2), np.arange(1, HD, 2)])
_SIGN_HEAD = np.concatenate([-np.ones(64, np.float32), np.ones(64, np.float32)])
_PERM_FULL = np.concatenate([h * HD + _PERM_HEAD for h in range(HEADS)])

# weight order inside the "wcat" input tensor
_WIDX = {"w_qa": 0, "w_qt": 1, "w_ka": 2, "w_kt": 3, "w_va": 4, "w_vt": 5,
         "w_o": 6, "w_ffn": 7}
# bias slots inside "bias_cat": per-partition [128, slot, ko]
_BIDX = {"b_qa": 0, "b_qt": 1, "b_ka": 2, "b_kt": 3, "b_o": 4, "b_ffn": 5}

USE_DIVIDE = False    # DVE tensor_tensor divide for softmax/LN normalize
USE_POOL_COS = True  # route some rope cos-muls to the Pool engine
USE_BCAST = True     # stride-0 broadcast rope tables (pair ropes)

_CACHED = None  # compiled Bass program, built once per process
LAST_RESULTS = None  # BassKernelResults of the most recent run


def _build_program():
    nc = bacc.Bacc("TRN2", target_bir_lowering=False, debug=False,
                   enable_asserts=False)

    xt_d = nc.dram_tensor("xt", (P, BPC, TK, T), BF, kind="ExternalInput").ap()
    xt8_d = nc.dram_tensor("xt8", (P, BPC, TK, T), F8D, kind="ExternalInput").ap()
    wq8_d = nc.dram_tensor("wq8", (3, P, TK, DIM), F8D, kind="ExternalInput").ap()
    hcat8_d = nc.dram_tensor("hcat8", (P, TK, 512), F8D, kind="ExternalInput").ap()
    wk8_d = nc.dram_tensor("wk8", (4, P, TK, DIM), F8D, kind="ExternalInput").ap()
    wcat_d = nc.dram_tensor("wcat", (8, P, TK, DIM), BF, kind="ExternalInput").ap()
    bias_d = nc.dram_tensor("bias_cat", (P, 6, TK), F32, kind="ExternalInput").ap()
    bv_d = nc.dram_tensor("bv_comb", (P, DIM), BF, kind="ExternalInput").ap()
    vsel_d = nc.dram_tensor("vsel", (P, P), BF, kind="ExternalInput").ap()
    cosq_d = nc.dram_tensor("cosq", (P, T), BF, kind="ExternalInput").ap()
    sinq_d = nc.dram_tensor("sinq", (P, T), BF, kind="ExternalInput").ap()
    cosk_d = nc.dram_tensor("cosk", (P, 2 * KV), BF, kind="ExternalInput").ap()
    sink_d = nc.dram_tensor("sink", (P, 2 * KV), BF, kind="ExternalInput").ap()
    out_d = nc.dram_tensor("outt", (P, BPC, TK, T), BF, kind="ExternalOutput").ap()

    with tile.TileContext(nc) as tc:
        _trace(nc, tc, xt_d, xt8_d, wq8_d, hcat8_d, wk8_d, wcat_d, bias_d,
               bv_d, vsel_d, cosq_d, sinq_d, cosk_d, sink_d, out_d)
    nc.compile()
    return nc


def _trace(nc, tc, xt_d, xt8_d, wq8_d, hcat8_d, wk8_d, wcat_d, bias_d,
           bv_d, vsel_d, cosq_d, sinq_d, cosk_d, sink_d, out_d):
    import contextlib
    ctx = contextlib.ExitStack()
    with ctx:
        consts = ctx.enter_context(tc.tile_pool(name="consts", bufs=1))
        acts = ctx.enter_context(tc.tile_pool(name="acts", bufs=1))
        qpool = ctx.enter_context(tc.tile_pool(name="qpool", bufs=4))
        wpool = ctx.enter_context(tc.tile_pool(name="wpool", bufs=1))
        wk8p = ctx.enter_context(tc.tile_pool(name="wk8p", bufs=2))
        swp = ctx.enter_context(tc.tile_pool(name="swp", bufs=3))
        q8p = ctx.enter_context(tc.tile_pool(name="q8p", bufs=2))
        sb512 = ctx.enter_context(tc.tile_pool(name="sb512", bufs=2))
        rcp_p = ctx.enter_context(tc.tile_pool(name="rcpp", bufs=1))
        psum = ctx.enter_context(tc.tile_pool(name="psum", bufs=4, space="PSUM"))
        pacc = ctx.enter_context(tc.tile_pool(name="pacc", bufs=2, space="PSUM"))

        def load_w(wname):
            wt = wpool.tile([P, TK, DIM], BF, tag="w", name=wname)
            nc.sync.dma_start(wt[:, :, :], wcat_d[_WIDX[wname], :, :, :])
            return wt

        # ---- DMAs in need order: the q_adapter b0 fp8 GEMM goes first so
        #      the PE starts within ~4us; everything else lands under it.
        # ---- early loads: whole-tensor DMAs (large linear transfers run
        # at 150-340 GB/s vs ~70 for 256KB chunks), one queue per tensor
        # in need order; everything early is resident by ~20us.
        bias_sb = consts.tile([P, 6, TK], F32, tag="bias")
        nc.scalar.dma_start(bias_sb[:], bias_d[:])
        cosq_sb = consts.tile([P, T], BF, tag="cosq")
        nc.scalar.dma_start(cosq_sb[:], cosq_d[:])
        sinq_sb = consts.tile([P, T], BF, tag="sinq")
        nc.scalar.dma_start(sinq_sb[:], sinq_d[:])
        wq8a = q8p.tile([P, TK, DIM], F8D, tag="q8", name="wq8a")
        nc.sync.dma_start(wq8a[:], wq8_d[0])
        xt8_sb = acts.tile([P, BPC, TK, T], F8D, tag="xt8")
        nc.gpsimd.dma_start(xt8_sb[:, 0], xt8_d[:, 0])
        hcat8 = consts.tile([P, TK, 512], F8D, tag="hcat8")
        nc.scalar.dma_start(hcat8[:], hcat8_d[:])
        wkt8 = wk8p.tile([P, TK, DIM], F8D, tag="wk8", name="wkt8")
        nc.scalar.dma_start(wkt8[:], wk8_d[0])
        wka8 = wk8p.tile([P, TK, DIM], F8D, tag="wk8", name="wka8")
        nc.sync.dma_start(wka8[:], wk8_d[1])
        wq8t = q8p.tile([P, TK, DIM], F8D, tag="q8", name="wq8t")
        nc.gpsimd.dma_start(wq8t[:], wq8_d[1])
        cosk_sb = consts.tile([P, 2 * KV], BF, tag="cosk")
        nc.scalar.dma_start(cosk_sb[:], cosk_d[:])
        sink_sb = consts.tile([P, 2 * KV], BF, tag="sink")
        nc.scalar.dma_start(sink_sb[:], sink_d[:])
        bv_sb = consts.tile([P, DIM], BF, tag="bv")
        nc.scalar.dma_start(bv_sb[:], bv_d[:])
        vsel = consts.tile([P, P], BF, tag="vsel")
        nc.scalar.dma_start(vsel[:], vsel_d[:])
        nc.gpsimd.dma_start(xt8_sb[:, 1], xt8_d[:, 1])
        ones_mat = consts.tile([P, P], BF, tag="onesm")
        nc.vector.memset(ones_mat[:], 1.0)
        eps_sb = consts.tile([P, 1], F32, tag="eps")
        nc.vector.memset(eps_sb[:], LN_EPS * OSC * OSC)

        def bias_ap(bname, n):
            return bias_sb[:, _BIDX[bname], n:n + 1]

        DR = mybir.MatmulPerfMode.DoubleRow

        def rope_q(dst, n, pool_cos=False):
            # dst: (128, TK, T) bf16, ropes tile n in place.
            # rotate_half: swap the two 64-partition blocks via 2 DMAs
            # issued from two idle queues; cos-mul optionally on Pool.
            sw = swp.tile([P, T], BF, tag="sw", name=f"sw{n}")
            sl = dst[:, n, :]
            nc.gpsimd.dma_start(sw[0:64, :], dst[64:128, n, :])
            nc.sync.dma_start(sw[64:128, :], dst[0:64, n, :])
            nc.vector.tensor_mul(sw[:], sw[:], sinq_sb[:])
            eng = nc.gpsimd if (pool_cos and USE_POOL_COS) else nc.vector
            eng.tensor_mul(sl, sl, cosq_sb[:])
            nc.vector.tensor_add(sl, sl, sw[:])

        def q_mm(qt_t, w8, bname, b, n, pool_cos=False):
            # fp8 DoubleRow: contract 2 k-tiles per pass (K=256 virtual)
            for c in range(NCH):
                cs = slice(c * 512, (c + 1) * 512)
                ps = psum.tile([P, 512], F32, tag="ps")
                for kp in range(TK // 2):
                    nc.tensor.matmul(
                        ps[:], w8[:, 2 * kp:2 * kp + 2, n * P:(n + 1) * P],
                        xt8_sb[:, b, 2 * kp:2 * kp + 2, cs],
                        start=(kp == 0), stop=(kp == TK // 2 - 1),
                        perf_mode=DR)
                nc.scalar.activation(
                    qt_t[:, n, cs], ps[:], AF.Identity,
                    bias=bias_ap(bname, n), scale=QSCALE)
            rope_q(qt_t, n, pool_cos=pool_cos)

        # ================= q_adapter b0 ================================
        q_rot = {}  # (qi, b) -> (128, TK, T) bf16, qi: 0=adapter 1=task
        qa0 = qpool.tile([P, TK, T], BF, tag="qbuf", name="qa0")
        q_rot[(0, 0)] = qa0
        for n in range(TK):
            q_mm(qa0, wq8a, "b_qa", 0, n)

        # ================= k projections ===============================
        # krot columns: [0:64]=task b0, [64:128]=task b1, [128:130]=ad b0,
        # [130:132]=ad b1
        krot = acts.tile([P, TK, 2 * KV], BF, tag="krot")
        for n in range(TK):
            ps = psum.tile([P, 512], F32, tag="ps")
            for kp in range(TK // 2):
                nc.tensor.matmul(ps[:, 0:128],
                                 wkt8[:, 2 * kp:2 * kp + 2, n * P:(n + 1) * P],
                                 hcat8[:, 2 * kp:2 * kp + 2, 0:128],
                                 start=(kp == 0), stop=(kp == TK // 2 - 1),
                                 perf_mode=DR)
            nc.scalar.activation(krot[:, n, 0:128], ps[:, 0:128],
                                 AF.Identity, bias=bias_ap("b_kt", n),
                                 scale=QSCALE)
        for n in range(TK):
            ps = psum.tile([P, 512], F32, tag="ps")
            for kp in range(TK // 2):
                nc.tensor.matmul(ps[:, 128:132],
                                 wka8[:, 2 * kp:2 * kp + 2, n * P:(n + 1) * P],
                                 hcat8[:, 2 * kp:2 * kp + 2, 192:196],
                                 start=(kp == 0), stop=(kp == TK // 2 - 1),
                                 perf_mode=DR)
            nc.scalar.activation(krot[:, n, 128:132], ps[:, 128:132],
                                 AF.Identity, bias=bias_ap("b_ka", n),
                                 scale=QSCALE)

        # ---- k rope (early: every attention score matmul waits on it) --
        cosk_b = cosk_sb[:].unsqueeze(1).broadcast_to([P, 2, 2 * KV])
        sink_b = sink_sb[:].unsqueeze(1).broadcast_to([P, 2, 2 * KV])
        for n in range(TK):
            sw = rcp_p.tile([P, 2 * KV], BF, tag="ksw", name=f"ksw{n}")
            sl = krot[:, n, :]
            nc.gpsimd.dma_start(sw[0:64, :], krot[64:128, n, :])
            nc.sync.dma_start(sw[64:128, :], krot[0:64, n, :])
            nc.vector.tensor_mul(sw[:], sw[:], sink_sb[:])
            nc.vector.tensor_mul(sl, sl, cosk_sb[:])
            nc.vector.tensor_add(sl, sl, sw[:])

        # v weights into the slots freed by wkt8/wka8 (their k-GEMM reads
        # are already complete here, so the issue does not block the queue);
        # the big bf16 residual tensors follow on the same Activation queue
        # (not needed until o_proj at ~100us)
        wvt8 = wk8p.tile([P, TK, DIM], F8D, tag="wk8", name="wvt8")
        nc.scalar.dma_start(wvt8[:], wk8_d[2])
        wva8 = wk8p.tile([P, TK, DIM], F8D, tag="wk8", name="wva8")
        nc.scalar.dma_start(wva8[:], wk8_d[3])
        xt_sb = acts.tile([P, BPC, TK, T], BF, tag="xt")
        nc.scalar.dma_start(xt_sb[:, 0], xt_d[:, 0])
        nc.scalar.dma_start(xt_sb[:, 1], xt_d[:, 1])

        # ================= q_task b0 ===================================
        qt0 = qpool.tile([P, TK, T], BF, tag="qbuf", name="qt0")
        q_rot[(1, 0)] = qt0
        for n in range(TK):
            q_mm(qt0, wq8t, "b_qt", 0, n)

        # ---- attention helpers ----------------------------------------
        attn = {}
        exs = {}

        def attn_scores(b, h):
            out = []
            for c in range(NCH):
                cs = slice(c * 512, (c + 1) * 512)
                scps = psum.tile([P, 512], F32, tag="ps")
                nc.tensor.matmul(scps[0:64, :], krot[:, h, b * 64:(b + 1) * 64],
                                 q_rot[(1, b)][:, h, cs], start=True, stop=True)
                nc.tensor.matmul(scps[64:66, :],
                                 krot[:, h, 128 + 2 * b:130 + 2 * b],
                                 q_rot[(0, b)][:, h, cs], start=True, stop=True)
                ex = sb512.tile([P, 512], BF, tag="ex", bufs=4, name="ex")
                nc.scalar.activation(ex[0:KV, :], scps[0:KV, :], AF.Exp)
                out.append(ex)
            exs[(b, h)] = out

        def attn_finish(b, h):
            at = attn[b]
            for c in range(NCH):
                cs = slice(c * 512, (c + 1) * 512)
                ex = exs[(b, h)][c]
                dnps = psum.tile([P, 512], F32, tag="ps")
                nc.tensor.matmul(dnps[:], ones_mat[0:KV, :],
                                 ex[0:KV, :], start=True, stop=True)
                ovps = psum.tile([P, 512], F32, tag="ps")
                nc.tensor.matmul(ovps[:], vcomb[0:KV, b, h * P:(h + 1) * P],
                                 ex[0:KV, :], start=True, stop=True)
                if USE_DIVIDE:
                    nc.vector.tensor_tensor(at[:, h, cs], ovps[:], dnps[:],
                                            OP.divide)
                else:
                    rcb = sb512.tile([P, 512], F32, tag="sdf", bufs=1,
                                     name="rcb")
                    nc.vector.reciprocal_approx_fast(rcb[:], dnps[:])
                    nc.vector.tensor_mul(at[:, h, cs], ovps[:], rcb[:])
            del exs[(b, h)]

        # ================= v projections (token-major) =================
        # vcomb rows: [0:64]=task tokens, [64:66]=adapter tokens; the bias
        # lands via a rank-2 matmul (vsel x bv) so the v pipeline only
        # depends on the PE + ScalarE.  Attention b0 scores for the first
        # two heads ride along (their ropes are already done).
        vcomb = acts.tile([P, BPC, DIM], BF, tag="vcomb")
        attn[0] = acts.tile([P, TK, T], F8D, tag="attn0", name="attn0")
        for b in range(BPC):
            for c in range(NCH):
                cs = slice(c * 512, (c + 1) * 512)
                ps = psum.tile([P, 512], F32, tag="ps")
                # bias first: rank-2 matmul starts the psum group over all
                # 128 rows (vsel is pre-scaled by XS8*WS8 on the host so
                # the shared 1/(XS8*WS8) consume scale cancels)
                nc.tensor.matmul(ps[:, :], vsel[0:2, :], bv_sb[0:2, cs],
                                 start=True, stop=False)
                for kp in range(TK // 2):
                    nc.tensor.matmul(ps[0:64, :],
                                     hcat8[:, 2 * kp:2 * kp + 2,
                                           b * 64:(b + 1) * 64],
                                     wvt8[:, 2 * kp:2 * kp + 2, cs],
                                     start=False, stop=False,
                                     perf_mode=DR, skip_group_check=True)
                # adapter last, 128 columns wide at tile_position (0,0)
                # (DR is only valid there): its 2 tokens sit at stationary
                # column base+64 (hcat8 col 192+2b) so they land on psum
                # rows 64:66; all other stationary columns are zero.  The
                # final pass carries the group stop over all 128 rows.
                for kp in range(TK // 2):
                    nc.tensor.matmul(ps[:, :],
                                     hcat8[:, 2 * kp:2 * kp + 2,
                                           128 + 2 * b:256 + 2 * b],
                                     wva8[:, 2 * kp:2 * kp + 2, cs],
                                     start=False, stop=(kp == TK // 2 - 1),
                                     perf_mode=DR,
                                     skip_group_check=(kp != TK // 2 - 1))
                nc.scalar.activation(vcomb[0:KV, b, cs], ps[0:KV, :],
                                     AF.Identity, scale=QSCALE * OSC)
            attn_scores(0, b)  # heads 0 and 1

        # ============ q_adapter b1 (x) attention b0 ====================
        qa1 = qpool.tile([P, TK, T], BF, tag="qbuf", name="qa1")
        q_rot[(0, 1)] = qa1
        for n in range(TK):
            q_mm(qa1, wq8a, "b_qa", 1, n, pool_cos=(n in (3, 7)))
            if 1 <= n <= 6:
                attn_scores(0, n + 1)
            if n >= 1:
                attn_finish(0, n - 1)
        attn_finish(0, TK - 1)

        wo8 = q8p.tile([P, TK, DIM], F8D, tag="q8", name="wo8")
        nc.sync.dma_start(wo8[:], wq8_d[2])  # into wq8a's slot (reads done)

        # ============ q_task b1 ========================================
        qt1 = qpool.tile([P, TK, T], BF, tag="qbuf", name="qt1")
        q_rot[(1, 1)] = qt1
        for n in range(TK):
            q_mm(qt1, wq8t, "b_qt", 1, n, pool_cos=(n in (3, 7)))

        # ============ o_proj b0 (x) attention b1 (x) LN stats b0 =======
        def oproj_tile(b, n, y):
            # fp8 DoubleRow o_proj; psum = OSC*WS8*(attn@wo), so scale by
            # 1/WS8 and add OSC*(x + b_o) (b_o folded into x on the host)
            for c in range(NCH):
                cs = slice(c * 512, (c + 1) * 512)
                ps = psum.tile([P, 512], F32, tag="ps")
                for kp in range(TK // 2):
                    nc.tensor.matmul(
                        ps[:], wo8[:, 2 * kp:2 * kp + 2, n * P:(n + 1) * P],
                        attn[b][:, 2 * kp:2 * kp + 2, cs],
                        start=(kp == 0), stop=(kp == TK // 2 - 1),
                        perf_mode=DR)
                nc.vector.scalar_tensor_tensor(
                    y[:, n, cs], ps[:], 1.0 / WS8,
                    xt_sb[:, b, n, cs], OP.mult, OP.add)

        def ln_stats_tile(sps, qps, y, n):
            for c in range(NCH):
                cs = slice(c * 512, (c + 1) * 512)
                ysq = sb512.tile([P, 512], BF, tag="s", name=f"ysq{n}{c}")
                nc.scalar.activation(ysq[:], y[:, n, cs], AF.Square)
                nc.tensor.matmul(sps[:, cs], ones_mat[:], y[:, n, cs],
                                 start=(n == 0), stop=(n == TK - 1),
                                 skip_group_check=True)
                nc.tensor.matmul(qps[:, cs], ones_mat[:], ysq[:],
                                 start=(n == 0), stop=(n == TK - 1),
                                 skip_group_check=True)

        attn[1] = acts.tile([P, TK, T], F8D, tag="attn1", name="attn1")
        y0 = qpool.tile([P, TK, T], BF, tag="qbuf", name="y0")
        sps0 = pacc.tile([P, T], F32, tag="acc", name="sps0")
        qps0 = pacc.tile([P, T], F32, tag="acc", name="qps0")
        for n in range(TK):
            oproj_tile(0, n, y0)
            if n <= 6:
                attn_scores(1, n)
            if n >= 1:
                attn_finish(1, n - 1)
            if n >= 1:
                ln_stats_tile(sps0, qps0, y0, n - 1)
        attn_scores(1, TK - 1)
        attn_finish(1, TK - 1)
        ln_stats_tile(sps0, qps0, y0, TK - 1)

        wffn = load_w("w_ffn")  # into slot freed by wva

        # ---- layernorm (folded: yn = (y - mu) / sdev) -----------------
        # split: ln_consume reads the psum stat accumulators (freeing them
        # for the next batch); ln_yn_tile normalizes one n-tile.
        ln_bc = {}

        def ln_consume(b, sps, qps):
            # mu_bc/sd_bc: (128, T) bf16, broadcast over partitions
            mu_bc = swp.tile([P, T], BF, tag="sw", name=f"mu{b}")
            sd_bc = swp.tile([P, T], BF, tag="sw", name=f"sd{b}")
            for c in range(NCH):
                cs = slice(c * 512, (c + 1) * 512)
                nc.vector.tensor_scalar_mul(mu_bc[:, cs], sps[:, cs], 1.0 / DIM)
                m2 = sb512.tile([P, 512], BF, tag="s", name=f"m2{b}{c}")
                nc.vector.tensor_mul(m2[:], mu_bc[:, cs], mu_bc[:, cs])
                vq = sb512.tile([P, 512], BF, tag="s", name=f"vq{b}{c}")
                nc.vector.scalar_tensor_tensor(
                    vq[:], qps[:, cs], 1.0 / DIM, m2[:], OP.mult, OP.subtract)
                if USE_DIVIDE:
                    nc.scalar.activation(sd_bc[:, cs], vq[:], AF.Sqrt,
                                         bias=eps_sb[:], scale=1.0)
                else:
                    sdf = sb512.tile([P, 512], F32, tag="sdf", bufs=1,
                                     name=f"sdf{b}{c}")
                    nc.scalar.activation(sdf[:], vq[:], AF.Sqrt,
                                         bias=eps_sb[:], scale=1.0)
                    nc.vector.reciprocal_approx_fast(sdf[:], sdf[:])
                    nc.vector.tensor_scalar_mul(sd_bc[:, cs], sdf[:], 1.0)
            ln_bc[b] = (mu_bc, sd_bc)

        def ln_yn_tile(b, y, yn, n):
            mu_bc, sd_bc = ln_bc[b]
            nc.vector.tensor_tensor(yn[:, n, :], y[:, n, :], mu_bc[:],
                                    OP.subtract)
            if USE_DIVIDE:
                nc.vector.tensor_tensor(yn[:, n, :], yn[:, n, :], sd_bc[:],
                                        OP.divide)
            else:
                nc.vector.tensor_tensor(yn[:, n, :], yn[:, n, :], sd_bc[:],
                                        OP.mult)

        def ffn_tile(b, n, yn):
            for c in range(NCH):
                cs = slice(c * 512, (c + 1) * 512)
                ps = psum.tile([P, 512], F32, tag="ps")
                for k in range(TK):
                    nc.tensor.matmul(ps[:], wffn[:, k, n * P:(n + 1) * P],
                                     yn[:, k, cs],
                                     start=(k == 0), stop=(k == TK - 1))
                ob = sb512.tile([P, 512], BF, tag="s", name=f"ob{b}{n}{c}")
                nc.scalar.activation(ob[:], ps[:], AF.Relu,
                                     bias=bias_ap("b_ffn", n), scale=1.0)
                nc.sync.dma_start(out_d[:, b, n, cs], ob[:])

        # ============ o_proj b1 (x) LN stats b1 (x) ln_apply(0) ========
        # ln_consume(0) first: it is the only reader of sps0/qps0, so the
        # pacc psum banks are free for sps1/qps1 right after.
        ln_consume(0, sps0, qps0)
        y1 = qpool.tile([P, TK, T], BF, tag="qbuf", name="y1")
        yn0 = qpool.tile([P, TK, T], BF, tag="qbuf", name="yn0")
        sps1 = pacc.tile([P, T], F32, tag="acc", name="sps1")
        qps1 = pacc.tile([P, T], F32, tag="acc", name="qps1")
        for n in range(TK):
            oproj_tile(1, n, y1)
            ln_yn_tile(0, y0, yn0, n)
            if n >= 1:
                ln_stats_tile(sps1, qps1, y1, n - 1)
        ln_stats_tile(sps1, qps1, y1, TK - 1)

        # ============ ln_apply(1), ffn b0 ==============================
        ln_consume(1, sps1, qps1)
        yn1 = qpool.tile([P, TK, T], BF, tag="qbuf", name="yn1")
        for n in range(TK):
            ln_yn_tile(1, y1, yn1, n)
            ffn_tile(0, n, yn0)

        # ============ ffn b1 ===========================================
        for n in range(TK):
            ffn_tile(1, n, yn1)


# =====================  host-side preparation  =========================

def _rope_tables(L):
    inv = 1.0 / (10000.0 ** (np.arange(0, HD, 2, dtype=np.float32) / HD))
    freqs = np.arange(L, dtype=np.float32)[:, None] * inv[None, :]
    emb = np.concatenate([freqs, freqs], axis=-1)  # (L, 128)
    return np.cos(emb), np.sin(emb)


def _perm_tables(L, scale):
    cos, sin = _rope_tables(L)  # (L, 128)
    cosP = (cos[:, _PERM_HEAD].T * scale).astype(np.float32)      # (128, L)
    sinN = (sin[:, _PERM_HEAD].T * _SIGN_HEAD[:, None] * scale).astype(np.float32)
    return cosP, sinN


def _w_sb(w, permute):
    # (1024 k, 1024 n) -> (128 p, 8 ko, 1024 n) bf16, optional column perm
    if permute:
        w = w[:, _PERM_FULL]
    return np.ascontiguousarray(
        w.reshape(TK, P, DIM).transpose(1, 0, 2)).astype(BF16)


def _b_slot(bvec, permute):
    if permute:
        bvec = bvec[_PERM_FULL]
    return bvec.reshape(TK, P).T  # (128, 8)


def kernel(**inputs):
    global _CACHED
    if _CACHED is None:
        _CACHED = _build_program()
    nc = _CACHED

    inp = {k: np.asarray(v) for k, v in inputs.items()}
    x = inp["x"].astype(np.float32)
    h_a = inp["h_a"].astype(np.float32)
    h_t = inp["h_t"].astype(np.float32)
    p_in = inp["p"].astype(np.float32)
    ratio = 1.0 / (1.0 + np.exp(-np.float32(inp["g"][0])))  # sigmoid

    # fold layernorm gamma/beta into the ffn weights
    w_ffn = inp["ln_g"].astype(np.float32)[:, None] * inp["w_ffn"].astype(np.float32)
    b_ffn = inp["b_ffn"].astype(np.float32) + (
        inp["ln_b"].astype(np.float32) @ inp["w_ffn"].astype(np.float32))

    # weights (shared across cores)
    wcat = np.stack([
        _w_sb(inp["w_qa"], True), _w_sb(inp["w_qt"], True),
        _w_sb(inp["w_ka"], True), _w_sb(inp["w_kt"], True),
        _w_sb(inp["w_va"], False), _w_sb(inp["w_vt"], False),
        _w_sb(inp["w_o"], False), _w_sb(w_ffn, False)])

    def _w8(w, permute):
        wp = w.astype(np.float32) * WS8
        if permute:
            wp = wp[:, _PERM_FULL]
        wp = np.clip(wp, -240.0, 240.0)
        return np.ascontiguousarray(
            wp.reshape(TK, P, DIM).transpose(1, 0, 2)).astype(FP8)

    wq8 = np.stack([_w8(inp["w_qa"], True), _w8(inp["w_qt"], True),
                    _w8(inp["w_o"], False)])
    wk8 = np.stack([_w8(inp["w_kt"], True), _w8(inp["w_ka"], True),
                    _w8(inp["w_vt"], False), _w8(inp["w_va"], False)])
    bias_cat = np.stack([
        _b_slot(inp["b_qa"], True), _b_slot(inp["b_qt"], True),
        _b_slot(inp["b_ka"], True), _b_slot(inp["b_kt"], True),
        _b_slot(inp["b_o"], False) * 0.0, _b_slot(b_ffn, False)],
        axis=1).astype(np.float32)  # (128, 6slots, 8ko)
    bv_comb = np.zeros((P, DIM), np.float32)
    bv_comb[0, :] = inp["b_vt"]
    bv_comb[1, :] = inp["b_va"]
    bv_comb = bv_comb.astype(BF16)
    # pre-scaled by XS8*WS8: cancels the fp8 consume scale on the v psum
    vsel = np.zeros((P, P), np.float32)
    vsel[0, 0:KT] = XS8 * WS8
    vsel[1, KT:KV] = XS8 * WS8
    vsel = vsel.astype(BF16)

    cosq, sinq = _perm_tables(T, np.float32(1.0 / math.sqrt(HD)))
    coskt, sinkt = _perm_tables(KT, ratio)
    coska, sinka = _perm_tables(KA, np.float32(1.0))
    cosk = np.concatenate([coskt, coskt, coska, coska], axis=1)  # (128, 132)
    sink = np.concatenate([sinkt, sinkt, sinka, sinka], axis=1)

    shared = {
        "wcat": wcat, "wq8": wq8, "wk8": wk8, "bias_cat": bias_cat,
        "bv_comb": bv_comb, "vsel": vsel,
        "cosq": cosq.astype(BF16), "sinq": sinq.astype(BF16),
        "cosk": cosk.astype(BF16), "sink": sink.astype(BF16),
    }

    in_maps = []
    for core in range(NCORES):
        b0 = core * BPC
        xc = x[b0:b0 + BPC]  # (2, 1024, 1024)
        xtf = np.ascontiguousarray(
            xc.reshape(BPC, T, TK, P).transpose(3, 0, 2, 1))  # (128,2,8,1024)
        xtr = xtf + inp["b_o"].astype(np.float32).reshape(TK, P).T[:, None, :, None]
        xt = (xtr * OSC).astype(BF16)  # o_proj residual: OSC*(x + b_o)
        xt8 = np.clip(xtf * XS8, -240.0, 240.0).astype(FP8)
        hcat8 = np.zeros((P, TK, 512), np.float32)
        for b in range(BPC):
            htT = h_t[b0 + b].T.reshape(TK, P, KT).transpose(1, 0, 2)
            hcat8[:, :, b * KT:(b + 1) * KT] = htT
            had = np.stack([h_a[b0 + b, 0], p_in[b0 + b, 0]], axis=1)  # (1024,2)
            hcat8[:, :, 192 + b * KA:192 + (b + 1) * KA] = (
                had.reshape(TK, P, KA).transpose(1, 0, 2))
        hcat8 = np.clip(hcat8 * XS8, -240.0, 240.0)
        in_maps.append({"xt": xt, "xt8": xt8, "hcat8": hcat8.astype(FP8),
                        **shared})

    res = run_bass_kernel_spmd(nc, in_maps, core_ids=list(range(NCORES)))
    global LAST_RESULTS
    LAST_RESULTS = res

    out = np.empty((B, T, DIM), np.float32)
    for core in range(NCORES):
        ot = res.results[core]["outt"]  # (128, 2, 8, 1024) bf16
        out[core * BPC:(core + 1) * BPC] = (
            ot.astype(np.float32).transpose(1, 3, 2, 0).reshape(BPC, T, DIM))
    return out
